# revision 1
# baseline (speedup 1.0000x reference)
"""Multi-head attention (B=4, T=2048, D=1024, H=16) on 8 Trainium2 cores.

Sharding: batch (4-way) x head-half (2-way) -> 8 cores.
Core c handles batch b = c//2 and heads g*8..g*8+8 where g = c%2.

Per-core device program (bf16 matmuls, fp32 psum accumulation):
  Stage A: kT[j,t] (k-part j-tiles) and v[t,j] GEMMs over the resident x^T.
    v is stored as [t, h, 65] with a ones column per head; biases are added
    via K=1 rank-1 matmuls into the psum accumulation groups.
  Stage B (interleaved): for each q j-tile jt, the qT GEMM runs interleaved
    with the attention of heads 2jt, 2jt+1 (the attention is exp/ACT-bound,
    so the qT matmuls fill PE idle slots; their psums share the AV psum
    slots via the pool tag).
  Attention per head: scoresT[ki,qi] = kT2^T qpad (K=128 with the other
    head's rows zeroed in qpad - keeps the PE activity monitor at full
    clock), exp (scale=1/8) straight from psum in 1024-wide ACT ops,
    AV: oT[j,qi] += [v|1|pad]^T wt with M=128 (row 64 = softmax denom).
    AV(k-1) is emitted after scores(k) (software pipeline).
  Normalize: chunk sums copied to 32-aligned partitions, one [128,512]
    parallel reciprocal, DRAM-bounce partition-broadcast, one DVE multiply.
  Out-proj: out[t,c] = ot^T @ woT over 4 j-tiles; output DMAs round-robin
    over two queues.

Host: transposes/reshapes inputs per core (bf16), sums the two head-half
partial outputs per batch, adds out_b.
"""

import numpy as np
import ml_dtypes
from contextlib import ExitStack

import concourse.bass as bass
import concourse.tile as tile
from concourse import bacc, mybir
from concourse.bass_utils import run_bass_kernel_spmd

BF16_NP = ml_dtypes.bfloat16

B, T, D = 4, 2048, 1024
H, HD = 16, 64
P = 128
NC = 8
HPC = 8          # heads per core
JC = HPC * HD    # 512 head-dim columns per core
KT = D // P      # 8 contraction tiles for QKV
TT = T // P      # 16 t tiles
TCH = T // 512   # 4 t chunks of 512
F32 = mybir.dt.float32
BF16 = mybir.dt.bfloat16

_cached = {}


def build_program():
    nc = bacc.Bacc("TRN2", target_bir_lowering=False, debug=False,
                   enable_asserts=True, num_devices=NC)

    xt_d = nc.dram_tensor("xt", [TCH, P, KT, 512], BF16, kind="ExternalInput").ap()
    wqk_d = nc.dram_tensor("wqk", [P, KT, 2 * JC], BF16, kind="ExternalInput").ap()
    wv_d = nc.dram_tensor("wv", [P, KT, JC], BF16, kind="ExternalInput").ap()
    bqk_d = nc.dram_tensor("bqk", [1, 2 * JC], BF16, kind="ExternalInput").ap()
    bv_d = nc.dram_tensor("bv", [1, JC], BF16, kind="ExternalInput").ap()
    ones_d = nc.dram_tensor("ones", [P, 512], BF16, kind="ExternalInput").ap()
    wo_d = nc.dram_tensor("wo", [P, JC // P, D], BF16, kind="ExternalInput").ap()
    out_d = nc.dram_tensor("out", [T, D], F32, kind="ExternalOutput").ap()

    EXP = mybir.ActivationFunctionType.Exp
    VW = HPC * (HD + 1)

    with tile.TileContext(nc) as tc:
        with ExitStack() as ctx:
            persist = ctx.enter_context(tc.tile_pool(name="persist", bufs=1))
            qk_sb = persist.tile([P, 2 * JC // P, T], BF16, tag="qk")
            # [t, 8 heads x [v(64)|ones(1)]] + 64 pad cols so the AV
            # stationary operand can be sliced 128 wide (M=128 keeps the PE
            # activity monitor from downclocking)
            vaug_f = persist.tile([P, TT, VW + HD], BF16, tag="vaug")
            ones2d = persist.tile([P, 512], BF16, tag="ones2d")
            bqk_sb = persist.tile([1, 2 * JC], BF16, tag="bqk")
            bv_sb = persist.tile([1, JC], BF16, tag="bv")
            xt_sb = persist.tile([P, TCH, KT, 512], BF16, tag="xt")
            wqk_sb = persist.tile([P, KT, 2 * JC], BF16, tag="wqk")
            ot_sb = persist.tile([P, JC // P, T], BF16, tag="ot")
            wo_sb = persist.tile([P, JC // P, D], BF16, tag="wo")

            # input DMAs: x chunks on the sync queue, weights on gpsimd
            for tci in range(TCH):
                nc.sync.dma_start(xt_sb[:, tci], xt_d[tci])
            for k in range(KT):
                nc.gpsimd.dma_start(wqk_sb[:, k, :], wqk_d[:, k, :])
            nc.sync.dma_start(ones2d[:], ones_d[:])
            nc.sync.dma_start(bqk_sb[:], bqk_d[:])
            nc.sync.dma_start(bv_sb[:], bv_d[:])
            nc.gpsimd.dma_start(wo_sb[:], wo_d[:])
            ones_sb = ones2d[0:1, :]

            vaug = vaug_f[:, :, 0:VW].rearrange(
                "p t (h e) -> p t h e", h=HPC)          # [128, 16, 8, 65]
            for tt in range(TT):
                nc.vector.tensor_copy(vaug[:, tt, :, HD:HD + 1],
                                      ones2d[:, 0:HPC, None])
                nc.vector.tensor_copy(vaug_f[:, tt, VW:VW + HD],
                                      ones2d[:, 0:HD])

            # ---------------- Stage A: kT and v GEMMs ----------------
            with ExitStack() as c1:
                with nc.named_scope("qkv_a"):
                    wvpool = c1.enter_context(tc.tile_pool(name="wvpool", bufs=1))
                    psA = c1.enter_context(
                        tc.tile_pool(name="psA", bufs=4, space="PSUM"))
                    wv_sb = wvpool.tile([P, KT, JC], BF16, tag="wv")
                    for k in range(KT):
                        nc.gpsimd.dma_start(wv_sb[:, k, :], wv_d[:, k, :])

                    for tci in range(TCH):
                        tsl = slice(tci * 512, (tci + 1) * 512)
                        # kT j-tiles (j = 4..7 of qk_sb)
                        for j in range(4):
                            ps = psA.tile([P, 512], F32, tag="psA",
                                          name=f"psk_{tci}_{j}")
                            for k in range(KT):
                                nc.tensor.matmul(
                                    ps[:],
                                    wqk_sb[:, k, JC + j * P:JC + (j + 1) * P],
                                    xt_sb[:, tci, k, :],
                                    start=(k == 0), stop=False)
                            nc.tensor.matmul(
                                ps[:],
                                bqk_sb[0:1, JC + j * P:JC + (j + 1) * P],
                                ones_sb[0:1, :],
                                start=False, stop=True)
                            nc.vector.tensor_copy(qk_sb[:, 4 + j, tsl], ps[:])
                        # v t-subtiles
                        for tt in range(4):
                            tglob = tci * 4 + tt
                            ps = psA.tile([P, 512], F32, tag="psA",
                                          name=f"psv_{tci}_{tt}")
                            for k in range(KT):
                                nc.tensor.matmul(
                                    ps[:],
                                    xt_sb[:, tci, k, tt * P:(tt + 1) * P],
                                    wv_sb[:, k, :],
                                    start=(k == 0), stop=False)
                            nc.tensor.matmul(
                                ps[:],
                                ones_sb[0:1, 0:P],
                                bv_sb[0:1, :],
                                start=False, stop=True)
                            nc.vector.tensor_copy(
                                vaug[:, tglob, :, 0:HD],
                                ps[:].rearrange("p (h d) -> p h d", h=HPC))

            # ------- Stage B: qT GEMM interleaved with attention -------
            with ExitStack() as c2:
                with nc.named_scope("attn"):
                    wtpool = c2.enter_context(tc.tile_pool(name="wtpool", bufs=3))
                    nrmpool = c2.enter_context(tc.tile_pool(name="nrmpool", bufs=2))
                    rbpool = c2.enter_context(tc.tile_pool(name="rbpool", bufs=2))
                    qpool = c2.enter_context(tc.tile_pool(name="qpool", bufs=2))
                    rdpool = c2.enter_context(
                        tc.tile_pool(name="rdpool", bufs=2, space="DRAM"))
                    pss = c2.enter_context(
                        tc.tile_pool(name="pss", bufs=2, space="PSUM"))
                    psav = c2.enter_context(
                        tc.tile_pool(name="psav", bufs=4, space="PSUM"))

                    # two rotating zero-padded qT buffers; heads alternate
                    # parity so each buffer's zero half stays zero
                    qpads = [qpool.tile([P, T], BF16, tag="qpad",
                                        name=f"qpad_{i}") for i in range(2)]
                    for i in range(2):
                        nc.vector.memset(qpads[i][:], 0.0)

                    def do_qt(jt):
                        # qT j-tile GEMM; psums share the "av" slots so the
                        # whole stage fits in 8 PSUM banks
                        for tci in range(TCH):
                            tsl = slice(tci * 512, (tci + 1) * 512)
                            ps = psav.tile([P, 512], F32, tag="av",
                                           name=f"psq_{jt}_{tci}")
                            for k in range(KT):
                                nc.tensor.matmul(
                                    ps[:],
                                    wqk_sb[:, k, jt * P:(jt + 1) * P],
                                    xt_sb[:, tci, k, :],
                                    start=(k == 0), stop=False)
                            nc.tensor.matmul(
                                ps[:],
                                bqk_sb[0:1, jt * P:(jt + 1) * P],
                                ones_sb[0:1, :],
                                start=False, stop=True)
                            nc.vector.tensor_copy(qk_sb[:, jt, tsl], ps[:])

                    def do_scores(h, k, qpad):
                        jt = h // 2
                        # full 128-row stationary operand (both heads' kT);
                        # the other head's rows hit the zero half of qpad, so
                        # the K=128 contraction equals K=64 but keeps the PE
                        # array fully active (HAM stays at 8/8)
                        kT2 = qk_sb[:, jt + 4, :]
                        wt = wtpool.tile([P, T], BF16, tag="wt",
                                         name=f"wt_{h}_{k}")
                        for half in range(2):
                            ps = pss.tile([P, 2, 512], F32, tag="ps_s",
                                          name=f"ps_s_{h}_{k}_{half}")
                            for cc in range(2):
                                c4 = half * 2 + cc
                                nc.tensor.matmul(
                                    ps[:, cc, :],
                                    kT2[:, k * P:(k + 1) * P],
                                    qpad[:, c4 * 512:(c4 + 1) * 512],
                                    start=True, stop=True)
                            nc.scalar.activation(
                                wt[:, half * 1024:(half + 1) * 1024],
                                ps[:].rearrange("p a b -> p (a b)"),
                                EXP, bias=0.0, scale=0.125)
                        return wt

                    def do_av(h, k, wt, av_tiles):
                        for c4 in range(4):
                            nc.tensor.matmul(
                                av_tiles[c4][:],
                                vaug_f[:, k, h * (HD + 1):h * (HD + 1) + P],
                                wt[:, c4 * 512:(c4 + 1) * 512],
                                start=(k == 0), stop=(k == TT - 1))

                    def finish_head(h, av_tiles):
                        pb = (h % 2) * 64
                        jt = h // 2
                        # free psum fast: copy o rows (unnormalized); chunk
                        # sums go to 32-aligned partitions so one [128,512]
                        # reciprocal covers all four chunks on parallel lanes
                        sums = nrmpool.tile([P, 512], F32, tag="sums",
                                            name=f"sums_{h}")
                        for c4 in range(4):
                            csl = slice(c4 * 512, (c4 + 1) * 512)
                            nc.vector.tensor_copy(
                                ot_sb[pb:pb + 64, jt, csl],
                                av_tiles[c4][0:HD, :])
                            nc.vector.tensor_copy(
                                sums[32 * c4:32 * c4 + 1, :],
                                av_tiles[c4][HD:HD + 1, :])
                        rcp = nrmpool.tile([P, 512], F32, tag="rcp",
                                           name=f"rcp_{h}")
                        nc.vector.reciprocal(rcp[:], sums[:])
                        rd = rdpool.tile([4, 512], F32, tag="rd",
                                         name=f"rd_{h}")
                        nc.sync.dma_start(rd[:], rcp[0:128:32, :])
                        rb = rbpool.tile([P, T], F32, tag="rb",
                                         name=f"rb_{h}")
                        rd_bcast = bass.AP(
                            tensor=rd.tensor, offset=rd.offset,
                            ap=[[0, 64], [512, 4], [1, 512]])
                        nc.sync.dma_start(
                            rb[pb:pb + 64, :].rearrange(
                                "p (c r) -> p c r", c=4),
                            rd_bcast)
                        nc.vector.tensor_mul(
                            ot_sb[pb:pb + 64, jt, :],
                            ot_sb[pb:pb + 64, jt, :],
                            rb[pb:pb + 64, :])

                    # software pipeline: AV(k-1) emitted after scores(k) so
                    # the PE always has independent matmuls between exp waits
                    prev = None
                    for h in range(HPC):
                        pb = (h % 2) * 64
                        jt = h // 2
                        if h % 2 == 0:
                            do_qt(jt)
                        qpad = qpads[h % 2]
                        nc.vector.tensor_copy(
                            qpad[pb:pb + HD, :], qk_sb[pb:pb + HD, jt, :])
                        av_tiles = [psav.tile([P, 512], F32, tag="av",
                                              name=f"av_{h}_{i}")
                                    for i in range(4)]
                        for k in range(TT):
                            wt = do_scores(h, k, qpad)
                            if prev is not None:
                                ph, pk, pwt, pav = prev
                                do_av(ph, pk, pwt, pav)
                                if pk == TT - 1:
                                    finish_head(ph, pav)
                            prev = (h, k, wt, av_tiles)
                    ph, pk, pwt, pav = prev
                    do_av(ph, pk, pwt, pav)
                    finish_head(ph, pav)

            # ---------------- Phase 3: out projection ----------------
            with ExitStack() as c3:
                with nc.named_scope("outproj"):
                    opool = c3.enter_context(tc.tile_pool(name="opool", bufs=3))
                    ps3 = c3.enter_context(
                        tc.tile_pool(name="ps3", bufs=4, space="PSUM"))

                    for tt in range(TT):
                        ost = opool.tile([P, D], F32, tag="ost")
                        for cc in range(2):
                            ps = ps3.tile([P, 512], F32, tag="pso")
                            for jt in range(JC // P):
                                nc.tensor.matmul(
                                    ps[:],
                                    ot_sb[:, jt, tt * P:(tt + 1) * P],
                                    wo_sb[:, jt, cc * 512:(cc + 1) * 512],
                                    start=(jt == 0), stop=(jt == JC // P - 1))
                            nc.vector.tensor_copy(
                                ost[:, cc * 512:(cc + 1) * 512], ps[:])
                        eng = nc.sync if tt % 2 == 0 else nc.gpsimd
                        eng.dma_start(out_d[tt * P:(tt + 1) * P, :], ost[:])

    nc.compile()
    return nc


def _prep_core_inputs(x, qkv_w, qkv_b, out_w, core):
    b, g = core // 2, core % 2
    jsl = slice(g * JC, (g + 1) * JC)

    xT = np.ascontiguousarray(x[b].T)                       # [1024, 2048]
    xt = np.ascontiguousarray(
        xT.reshape(KT, P, TCH, 512).transpose(2, 1, 0, 3))  # [4, 128, 8, 512]

    Wq = qkv_w[0 * D:1 * D][jsl]                            # [512, 1024]
    Wk = qkv_w[1 * D:2 * D][jsl]
    Wv = qkv_w[2 * D:3 * D][jsl]
    WqkT = np.concatenate([Wq, Wk], axis=0).T               # [1024, 1024]
    wqk = np.ascontiguousarray(
        WqkT.reshape(KT, P, 2 * JC).transpose(1, 0, 2))     # [128, 8, 1024]
    WvT = Wv.T                                              # [1024, 512]
    wv = np.ascontiguousarray(
        WvT.reshape(KT, P, JC).transpose(1, 0, 2))          # [128, 8, 512]

    bqk = np.concatenate(
        [qkv_b[0 * D:1 * D][jsl], qkv_b[1 * D:2 * D][jsl]])[None, :]
    bv = qkv_b[2 * D:3 * D][jsl][None, :]

    WoT = np.ascontiguousarray(out_w[:, jsl].T)             # [512, 1024]
    wo = np.ascontiguousarray(
        WoT.reshape(JC // P, P, D).transpose(1, 0, 2))      # [128, 4, 1024]

    return {
        "xt": xt.astype(BF16_NP),
        "wqk": wqk.astype(BF16_NP),
        "wv": wv.astype(BF16_NP),
        "bqk": bqk.astype(BF16_NP),
        "bv": bv.astype(BF16_NP),
        "wo": wo.astype(BF16_NP),
        "ones": np.ones((P, 512), dtype=BF16_NP),
    }


def run(x, qkv_w, qkv_b, out_w, out_b, trace=False, tmpdir=None):
    if "nc" not in _cached:
        _cached["nc"] = build_program()
    nc = _cached["nc"]
    in_maps = [_prep_core_inputs(x, qkv_w, qkv_b, out_w, c) for c in range(NC)]
    res = run_bass_kernel_spmd(nc, in_maps, core_ids=list(range(NC)),
                               trace=trace, tmpdir=tmpdir)
    parts = np.stack([res.results[c]["out"] for c in range(NC)])  # [8, T, D]
    out = parts.reshape(B, 2, T, D).sum(axis=1) + out_b[None, None, :]
    return out.astype(np.float32), res


def kernel(x, qkv_w, qkv_b, out_w, out_b):
    x = np.asarray(x, dtype=np.float32)
    qkv_w = np.asarray(qkv_w, dtype=np.float32)
    qkv_b = np.asarray(qkv_b, dtype=np.float32)
    out_w = np.asarray(out_w, dtype=np.float32)
    out_b = np.asarray(out_b, dtype=np.float32)
    out, _ = run(x, qkv_w, qkv_b, out_w, out_b, trace=False)
    return out



# revision 12
# speedup vs baseline: 1.1134x; 1.1134x over previous
"""Multi-head attention (B=4, T=2048, D=1024, H=16) on 8 Trainium2 cores.

Sharding: batch (4-way) x head-half (2-way) -> 8 cores.
Core c handles batch b = c//2 and heads g*8..g*8+8 where g = c%2.

v2: single fully-pipelined instruction stream.
  - Attention (scores -> exp -> AV) starts at ~12us; the k/q/v projection
    GEMMs that are not needed immediately run as "filler" matmuls woven
    into the attention stream so the PE never idles while the scalar
    engine (exp) is the per-iteration rate limiter.
  - q/k projections optionally run as fp8e4 DoubleRow matmuls (2 k-tiles
    per instruction, 2x effective contraction rate). Inputs are scaled
    (x*16, W*64) into the fp8 normal range; the 1/1024 fixup is folded
    into the psum->SBUF evacuation. v stays bf16 (its quantization error
    would land directly on the output).
  - q/k/v biases are folded into the psum evacuation (DVE tensor_scalar /
    tensor_tensor) instead of rank-1 matmuls.
  - AV accumulates per (head, half-of-T) in a single 2-bank psum tile;
    scores psum 2x2 banks; 2 banks left for the filler GEMMs. Softmax
    denominators (ones-column of the augmented v) are evacuated by the
    scalar engine (copy shares the act table with exp), o rows by DVE.
  - Normalization per (head, half): reciprocal_approx_fast + DRAM-bounce
    partition broadcast + one DVE multiply. Out-projection tiles 0-7 run
    inside the last head's second-half window; tiles 8-15 after it.
  - Output is bf16 (halves DMA); host sums the two head-half partials.

Host: transposes/reshapes inputs per core (bf16/fp8), sums partials,
adds out_b.
"""

import numpy as np
import ml_dtypes
from contextlib import ExitStack

import concourse.bass as bass
import concourse.tile as tile
from concourse import bacc, mybir
from concourse.bass_utils import run_bass_kernel_spmd

BF16_NP = ml_dtypes.bfloat16
FP8_NP = ml_dtypes.float8_e4m3

B, T, D = 4, 2048, 1024
H, HD = 16, 64
P = 128
NC = 8
HPC = 8          # heads per core
JC = HPC * HD    # 512 head-dim columns per core
KT = D // P      # 8 contraction tiles for QKV
TT = T // P      # 16 t tiles
TCH = T // 512   # 4 t chunks of 512
F32 = mybir.dt.float32
BF16 = mybir.dt.bfloat16
FP8 = mybir.dt.float8e4

USE_FP8_KQ = True
X8_SCALE = 16.0
W8_SCALE = 64.0
KQ_FIX = 1.0 / (X8_SCALE * W8_SCALE)

_cached = {}


def build_program():
    nc = bacc.Bacc("TRN2", target_bir_lowering=False, debug=False,
                   enable_asserts=True, num_devices=NC)

    xt16_d = nc.dram_tensor("xt16", [TCH, P, KT, 512], BF16,
                            kind="ExternalInput").ap()
    if USE_FP8_KQ:
        xt8_d = nc.dram_tensor("xt8", [TCH, P, KT // 2, 2, 512], FP8,
                               kind="ExternalInput").ap()
        wqk_d = nc.dram_tensor("wqk", [P, 8, KT // 2, 2, P], FP8,
                               kind="ExternalInput").ap()
    else:
        wqk_d = nc.dram_tensor("wqk", [P, KT, 2 * JC], BF16,
                               kind="ExternalInput").ap()
    wv_d = nc.dram_tensor("wv", [P, KT, JC], BF16, kind="ExternalInput").ap()
    wo_d = nc.dram_tensor("wo", [P, JC // P, D], BF16,
                          kind="ExternalInput").ap()
    bqk_d = nc.dram_tensor("bqk", [P, 8], F32, kind="ExternalInput").ap()
    bvb_d = nc.dram_tensor("bvb", [P, JC], BF16, kind="ExternalInput").ap()
    out_d = nc.dram_tensor("out", [T, D], BF16, kind="ExternalOutput").ap()

    EXP = mybir.ActivationFunctionType.Exp
    COPY = mybir.ActivationFunctionType.Copy
    DR = mybir.MatmulPerfMode.DoubleRow
    SC = 0.125  # 1/sqrt(HD)

    with tile.TileContext(nc) as tc:
        with ExitStack() as ctx:
            persist = ctx.enter_context(tc.tile_pool(name="persist", bufs=1))
            xt16_sb = persist.tile([P, TCH, KT, 512], BF16, tag="xt16")
            if USE_FP8_KQ:
                xt8_sb = persist.tile([P, TCH, KT // 2, 2, 512], FP8,
                                      tag="xt8")
                wqk_sb = persist.tile([P, 8, KT // 2, 2, P], FP8, tag="wqk")
            else:
                wqk_sb = persist.tile([P, KT, 2 * JC], BF16, tag="wqk")
            wv_sb = persist.tile([P, KT, JC], BF16, tag="wv")
            wo_sb = persist.tile([P, JC // P, D], BF16, tag="wo")
            bqk_sb = persist.tile([P, 8], F32, tag="bqk")
            bvb_sb = persist.tile([P, JC], BF16, tag="bvb")
            qk_sb = persist.tile([P, 8, T], BF16, tag="qk")
            # [t, 8 x [v(64)|1]] + 64 pad cols so the AV stationary operand
            # can be sliced 128 wide
            VW = HPC * (HD + 1)
            vaug_f = persist.tile([P, TT, VW + HD], BF16, tag="vaug")
            ot_sb = persist.tile([P, JC // P, T], BF16, tag="ot")

            # ---- input DMAs (program order = queue order) ----
            if USE_FP8_KQ:
                for tci in range(TCH):
                    nc.sync.dma_start(xt8_sb[:, tci], xt8_d[tci])
            for tci in range(TCH):
                nc.sync.dma_start(xt16_sb[:, tci], xt16_d[tci])
            nc.gpsimd.dma_start(wqk_sb[:], wqk_d[:])
            nc.gpsimd.dma_start(wv_sb[:], wv_d[:])
            nc.gpsimd.dma_start(bqk_sb[:], bqk_d[:])
            nc.gpsimd.dma_start(bvb_sb[:], bvb_d[:])
            nc.gpsimd.dma_start(wo_sb[:], wo_d[:])

            vaug = vaug_f[:, :, 0:VW].rearrange(
                "p t (h e) -> p t h e", h=HPC)          # [128, 16, 8, 65]
            for tt in range(TT):
                nc.vector.memset(vaug[:, tt, :, HD:HD + 1], 1.0)
                nc.vector.memset(vaug_f[:, tt, VW:VW + HD], 0.0)

            wtpool = ctx.enter_context(tc.tile_pool(name="wtpool", bufs=3))
            qpool = ctx.enter_context(tc.tile_pool(name="qpool", bufs=2))
            nrmpool = ctx.enter_context(tc.tile_pool(name="nrmpool", bufs=2))
            rcpool = ctx.enter_context(tc.tile_pool(name="rcpool", bufs=2))
            rbpool = ctx.enter_context(tc.tile_pool(name="rbpool", bufs=2))
            ostpool = ctx.enter_context(tc.tile_pool(name="ostpool", bufs=3))
            rdpool = ctx.enter_context(
                tc.tile_pool(name="rdpool", bufs=2, space="DRAM"))
            pss = ctx.enter_context(
                tc.tile_pool(name="pss", bufs=2, space="PSUM"))
            avp = ctx.enter_context(
                tc.tile_pool(name="avp", bufs=1, space="PSUM"))
            psf = ctx.enter_context(
                tc.tile_pool(name="psf", bufs=2, space="PSUM"))

            qpads = [qpool.tile([P, T], BF16, tag="qpad",
                                name=f"qpad_{i}") for i in range(2)]
            for i in range(2):
                nc.vector.memset(qpads[i][:], 0.0)

            # ---------------- filler group builders ----------------
            # Each group is a list of closures; each closure emits one PE
            # matmul (the last also emits the psum evacuation on DVE).

            def kq_group(jcol, tci):
                """qk_sb j-tile jcol (0-3 = q j, 4-7 = kT j) over t-chunk."""
                wcol = jcol * P if jcol < 4 else JC + (jcol - 4) * P
                tsl = slice(tci * 512, (tci + 1) * 512)
                steps = []
                box = {}
                nsteps = KT // 2 if USE_FP8_KQ else KT

                def mk(i):
                    first, last = i == 0, i == nsteps - 1

                    def step():
                        if first:
                            box["ps"] = psf.tile([P, 512], F32, tag="psf",
                                                 name=f"kq_{jcol}_{tci}")
                        if USE_FP8_KQ:
                            nc.tensor.matmul(
                                box["ps"][:],
                                wqk_sb[:, jcol, i],
                                xt8_sb[:, tci, i],
                                start=first, stop=last, perf_mode=DR)
                        else:
                            nc.tensor.matmul(
                                box["ps"][:],
                                wqk_sb[:, i, wcol:wcol + P],
                                xt16_sb[:, tci, i],
                                start=first, stop=last)
                        if last:
                            if USE_FP8_KQ:
                                nc.vector.tensor_scalar(
                                    qk_sb[:, jcol, tsl], box["ps"][:],
                                    KQ_FIX, bqk_sb[:, jcol:jcol + 1],
                                    op0=mybir.AluOpType.mult,
                                    op1=mybir.AluOpType.add)
                            else:
                                nc.vector.tensor_scalar(
                                    qk_sb[:, jcol, tsl], box["ps"][:],
                                    bqk_sb[:, jcol:jcol + 1], None,
                                    op0=mybir.AluOpType.add)
                    return step
                for i in range(nsteps):
                    steps.append(mk(i))
                return steps

            def v_group(tglob):
                tci, tt = tglob // 4, tglob % 4
                steps = []
                box = {}

                def mk(k):
                    first, last = k == 0, k == KT - 1

                    def step():
                        if first:
                            box["ps"] = psf.tile([P, 512], F32, tag="psf",
                                                 name=f"v_{tglob}")
                        nc.tensor.matmul(
                            box["ps"][:],
                            xt16_sb[:, tci, k, tt * P:(tt + 1) * P],
                            wv_sb[:, k, :],
                            start=first, stop=last)
                        if last:
                            nc.vector.tensor_tensor(
                                vaug[:, tglob, :, 0:HD],
                                box["ps"][:].rearrange(
                                    "p (h d) -> p h d", h=HPC),
                                bvb_sb[:].rearrange("p (h d) -> p h d", h=HPC),
                                op=mybir.AluOpType.add)
                    return step
                for k in range(KT):
                    steps.append(mk(k))
                return steps

            ost_box = {}

            def outproj_group(tt, cc):
                steps = []
                box = {}

                def mk(jt):
                    first, last = jt == 0, jt == JC // P - 1

                    def step():
                        if first:
                            box["ps"] = psf.tile([P, 512], F32, tag="psf",
                                                 name=f"op_{tt}_{cc}")
                            if cc == 0:
                                ost_box[tt] = ostpool.tile(
                                    [P, D], BF16, tag="ost", name=f"ost_{tt}")
                        nc.tensor.matmul(
                            box["ps"][:],
                            ot_sb[:, jt, tt * P:(tt + 1) * P],
                            wo_sb[:, jt, cc * 512:(cc + 1) * 512],
                            start=first, stop=last)
                        if last:
                            nc.vector.tensor_copy(
                                ost_box[tt][:, cc * 512:(cc + 1) * 512],
                                box["ps"][:])
                            if cc == 1:
                                eng = nc.sync if tt % 2 == 0 else nc.gpsimd
                                eng.dma_start(out_d[tt * P:(tt + 1) * P, :],
                                              ost_box[tt][:])
                    return step
                for jt in range(JC // P):
                    steps.append(mk(jt))
                return steps

            # ---------------- preamble ----------------
            for tci in range(TCH):
                for s in kq_group(4, tci):   # kT j0 tile (heads 0,1 k)
                    s()
                for s in kq_group(0, tci):   # q j0 tile (heads 0,1 q)
                    s()
            for tg in range(4):
                for s in v_group(tg):
                    s()
            nc.vector.tensor_copy(qpads[0][0:HD, :], qk_sb[0:HD, 0, :])

            # ---------------- filler window assignments ----------------
            # win key = (h, half); each window spans 16 attention iters.
            win_steps = {}
            win_start = {}
            win_len = {}

            def assign(windows, groups, start=0, length=16):
                flat = [s for g in groups for s in g]
                n = len(windows)
                for i, w in enumerate(windows):
                    win_steps[w] = flat[len(flat) * i // n:
                                        len(flat) * (i + 1) // n]
                    win_start[w] = start
                    win_len[w] = length

            # v tiles 4..15 must land just ahead of AV(h0, half0, k)
            assign([(0, 0)], [v_group(tg) for tg in range(4, 16)], length=14)
            assign([(0, 1), (1, 0)],
                   [kq_group(5, tci) for tci in range(TCH)]
                   + [kq_group(1, tci) for tci in range(TCH)])
            assign([(1, 1), (2, 0)],
                   [kq_group(6, tci) for tci in range(TCH)]
                   + [kq_group(2, tci) for tci in range(TCH)])
            assign([(2, 1), (3, 0), (3, 1), (4, 0)],
                   [kq_group(7, tci) for tci in range(TCH)]
                   + [kq_group(3, tci) for tci in range(TCH)])
            assign([(7, 1)],
                   [outproj_group(tt, cc) for tt in range(8)
                    for cc in range(2)], start=4, length=12)

            # ---------------- attention stream ----------------
            prev = None          # (h, half, k, wt)
            af_tiles = {}
            sums_tiles = {}

            def emit_av(ph, phalf, pk, pwt):
                if pk == 0:
                    af_tiles[(ph, phalf)] = avp.tile(
                        [P, 2, 512], F32, tag="av", name=f"af_{ph}_{phalf}")
                paf = af_tiles[(ph, phalf)]
                for c in range(2):
                    nc.tensor.matmul(
                        paf[:, c, :],
                        vaug_f[:, pk, ph * (HD + 1):ph * (HD + 1) + P],
                        pwt[:, c * 512:(c + 1) * 512],
                        start=(pk == 0), stop=(pk == TT - 1))

            def emit_evac_finish(ph, phalf):
                paf = af_tiles.pop((ph, phalf))
                pb = (ph % 2) * 64
                jt = ph // 2
                hsl = slice(phalf * 1024, (phalf + 1) * 1024)
                if phalf == 0:
                    sums_tiles[ph] = nrmpool.tile([P, 512], F32, tag="sums",
                                                  name=f"sums_{ph}")
                    nc.vector.memset(sums_tiles[ph][:], 1.0)
                sums = sums_tiles[ph]
                # o rows on DVE; denominator row via scalar engine (copy
                # shares the exp act table -> no table reload)
                nc.vector.tensor_copy(
                    ot_sb[pb:pb + HD, jt, hsl],
                    paf[0:HD].rearrange("p a b -> p (a b)"))
                for c in range(2):
                    cglob = 2 * phalf + c
                    nc.scalar.activation(
                        sums[32 * cglob:32 * cglob + 1, :],
                        paf[HD:HD + 1, c, :], COPY)
                # normalization for this half: chunk sums live at 32-aligned
                # partitions (engine partition bases must be 32-aligned); the
                # reciprocal runs on the contiguous 64-partition span and the
                # strided DMA picks out the two valid rows.
                pbase = 64 * phalf
                rcp = rcpool.tile([P, 512], F32, tag="rcp",
                                  name=f"rcp_{ph}_{phalf}")
                nc.vector.reciprocal_approx_fast(rcp[:], sums[:])
                rd = rdpool.tile([2, 512], F32, tag="rd",
                                 name=f"rd_{ph}_{phalf}")
                nc.sync.dma_start(rd[:], rcp[pbase:pbase + 33:32, :])
                rb = rbpool.tile([P, 1024], F32, tag="rb",
                                 name=f"rb_{ph}_{phalf}")
                rd_bcast = bass.AP(
                    tensor=rd.tensor, offset=rd.offset,
                    ap=[[0, 64], [512, 2], [1, 512]])
                nc.sync.dma_start(
                    rb[pb:pb + 64, :].rearrange("p (c r) -> p c r", c=2),
                    rd_bcast)
                nc.vector.tensor_mul(
                    ot_sb[pb:pb + 64, jt, hsl],
                    ot_sb[pb:pb + 64, jt, hsl],
                    rb[pb:pb + 64, :])

            for h in range(HPC):
                jt = h // 2
                qpad = qpads[h % 2]
                kT2 = qk_sb[:, 4 + jt, :]
                for half in range(2):
                    w = (h, half)
                    steps = win_steps.get(w, [])
                    s0 = win_start.get(w, 0)
                    slen = win_len.get(w, 16)
                    nsteps = len(steps)
                    emitted = 0
                    for k in range(TT):
                        # scores for (h, half, k)
                        ps = pss.tile([P, 2, 512], F32, tag="pss",
                                      name=f"s_{h}_{half}_{k}")
                        for c in range(2):
                            nc.tensor.matmul(
                                ps[:, c, :],
                                kT2[:, k * P:(k + 1) * P],
                                qpad[:, half * 1024 + c * 512:
                                     half * 1024 + (c + 1) * 512],
                                start=True, stop=True)
                        # filler slice for this iteration
                        if nsteps and k >= s0:
                            tgt = min(nsteps,
                                      -(-nsteps * (k - s0 + 1) // slen))
                            while emitted < tgt:
                                steps[emitted]()
                                emitted += 1
                        wt = wtpool.tile([P, 1024], BF16, tag="wt",
                                         name=f"wt_{h}_{half}_{k}")
                        nc.scalar.activation(
                            wt[:], ps[:].rearrange("p a b -> p (a b)"),
                            EXP, bias=0.0, scale=SC)
                        if prev is not None:
                            emit_av(*prev)
                            if prev[2] == TT - 1:
                                emit_evac_finish(prev[0], prev[1])
                        prev = (h, half, k, wt)
                        # prefetch next head's qpad late in this head
                        if half == 1 and k == 13 and h + 1 < HPC:
                            nh = h + 1
                            npb = (nh % 2) * 64
                            nc.vector.tensor_copy(
                                qpads[nh % 2][npb:npb + HD, :],
                                qk_sb[npb:npb + HD, nh // 2, :])
                    while emitted < nsteps:
                        steps[emitted]()
                        emitted += 1

            # drain: last AV + finish + outproj tiles 8..15
            emit_av(*prev)
            emit_evac_finish(prev[0], prev[1])
            for tt in range(8, 16):
                for cc in range(2):
                    for s in outproj_group(tt, cc):
                        s()

    nc.compile()
    return nc


def _prep_core_inputs(x, qkv_w, qkv_b, out_w, core):
    b, g = core // 2, core % 2
    jsl = slice(g * JC, (g + 1) * JC)

    xT = np.ascontiguousarray(x[b].T)                       # [1024, 2048]
    xk = xT.reshape(KT, P, TCH, 512)
    xt16 = np.ascontiguousarray(xk.transpose(2, 1, 0, 3))   # [4,128,8,512]

    Wq = qkv_w[0 * D:1 * D][jsl]                            # [512, 1024]
    Wk = qkv_w[1 * D:2 * D][jsl]
    Wv = qkv_w[2 * D:3 * D][jsl]
    WqkT = np.concatenate([Wq, Wk], axis=0).T               # [1024, 1024]
    WvT = Wv.T                                              # [1024, 512]
    wv = np.ascontiguousarray(
        WvT.reshape(KT, P, JC).transpose(1, 0, 2))          # [128, 8, 512]

    bq = qkv_b[0 * D:1 * D][jsl]
    bk = qkv_b[1 * D:2 * D][jsl]
    bv = qkv_b[2 * D:3 * D][jsl]
    bqk = np.ascontiguousarray(
        np.concatenate([bq, bk]).reshape(8, P).T)           # [128, 8]
    bvb = np.ascontiguousarray(np.tile(bv[None, :], (P, 1)))

    WoT = np.ascontiguousarray(out_w[:, jsl].T)             # [512, 1024]
    wo = np.ascontiguousarray(
        WoT.reshape(JC // P, P, D).transpose(1, 0, 2))      # [128, 4, 1024]

    inputs = {
        "xt16": xt16.astype(BF16_NP),
        "wv": wv.astype(BF16_NP),
        "wo": wo.astype(BF16_NP),
        "bqk": bqk.astype(np.float32),
        "bvb": bvb.astype(BF16_NP),
    }
    if USE_FP8_KQ:
        xt8 = np.ascontiguousarray(
            (xk * X8_SCALE).reshape(KT // 2, 2, P, TCH, 512)
            .transpose(3, 2, 0, 1, 4))                      # [4,128,4,2,512]
        wqk8 = np.ascontiguousarray(
            (WqkT * W8_SCALE).reshape(KT // 2, 2, P, 8, P)
            .transpose(2, 3, 0, 1, 4))                      # [128,8,4,2,128]
        inputs["xt8"] = xt8.astype(FP8_NP)
        inputs["wqk"] = wqk8.astype(FP8_NP)
    else:
        wqk = np.ascontiguousarray(
            WqkT.reshape(KT, P, 2 * JC).transpose(1, 0, 2))  # [128, 8, 1024]
        inputs["wqk"] = wqk.astype(BF16_NP)
    return inputs


def run(x, qkv_w, qkv_b, out_w, out_b, trace=False, tmpdir=None):
    if "nc" not in _cached:
        _cached["nc"] = build_program()
    nc = _cached["nc"]
    in_maps = [_prep_core_inputs(x, qkv_w, qkv_b, out_w, c) for c in range(NC)]
    res = run_bass_kernel_spmd(nc, in_maps, core_ids=list(range(NC)),
                               trace=trace, tmpdir=tmpdir)
    parts = np.stack([np.asarray(res.results[c]["out"], dtype=np.float32)
                      for c in range(NC)])                  # [8, T, D]
    out = parts.reshape(B, 2, T, D).sum(axis=1) + out_b[None, None, :]
    return out.astype(np.float32), res


def kernel(x, qkv_w, qkv_b, out_w, out_b):
    x = np.asarray(x, dtype=np.float32)
    qkv_w = np.asarray(qkv_w, dtype=np.float32)
    qkv_b = np.asarray(qkv_b, dtype=np.float32)
    out_w = np.asarray(out_w, dtype=np.float32)
    out_b = np.asarray(out_b, dtype=np.float32)
    out, _ = run(x, qkv_w, qkv_b, out_w, out_b, trace=False)
    return out


if __name__ == "__main__":
    import jax
    import reference
    inputs = {k: np.asarray(v) for k, v in reference.setup_inputs().items()}
    expected = np.asarray(reference.reference(**inputs))
    got = kernel(**inputs)
    err = np.linalg.norm(got - expected) / np.linalg.norm(expected)
    print("Relative error:", err)


# revision 15
# speedup vs baseline: 1.1354x; 1.0197x over previous
"""Multi-head attention (B=4, T=2048, D=1024, H=16) on 8 Trainium2 cores.

Sharding: batch (4-way) x head-half (2-way) -> 8 cores.
Core c handles batch b = c//2 and heads g*8..g*8+8 where g = c%2.

v2: single fully-pipelined instruction stream.
  - Attention (scores -> exp -> AV) starts at ~12us; the k/q/v projection
    GEMMs that are not needed immediately run as "filler" matmuls woven
    into the attention stream so the PE never idles while the scalar
    engine (exp) is the per-iteration rate limiter.
  - q/k projections optionally run as fp8e4 DoubleRow matmuls (2 k-tiles
    per instruction, 2x effective contraction rate). Inputs are scaled
    (x*16, W*64) into the fp8 normal range; the 1/1024 fixup is folded
    into the psum->SBUF evacuation. v stays bf16 (its quantization error
    would land directly on the output).
  - q/k/v biases are folded into the psum evacuation (DVE tensor_scalar /
    tensor_tensor) instead of rank-1 matmuls.
  - AV accumulates per (head, half-of-T) in a single 2-bank psum tile;
    scores psum 2x2 banks; 2 banks left for the filler GEMMs. Softmax
    denominators (ones-column of the augmented v) are evacuated by the
    scalar engine (copy shares the act table with exp), o rows by DVE.
  - Normalization per (head, half): reciprocal_approx_fast + DRAM-bounce
    partition broadcast + one DVE multiply. Out-projection tiles 0-7 run
    inside the last head's second-half window; tiles 8-15 after it.
  - Output is bf16 (halves DMA); host sums the two head-half partials.

Host: transposes/reshapes inputs per core (bf16/fp8), sums partials,
adds out_b.
"""

import numpy as np
import ml_dtypes
from contextlib import ExitStack

import concourse.bass as bass
import concourse.tile as tile
from concourse import bacc, mybir
from concourse.bass_utils import run_bass_kernel_spmd

BF16_NP = ml_dtypes.bfloat16
FP8_NP = ml_dtypes.float8_e4m3

B, T, D = 4, 2048, 1024
H, HD = 16, 64
P = 128
NC = 8
HPC = 8          # heads per core
JC = HPC * HD    # 512 head-dim columns per core
KT = D // P      # 8 contraction tiles for QKV
TT = T // P      # 16 t tiles
TCH = T // 512   # 4 t chunks of 512
F32 = mybir.dt.float32
BF16 = mybir.dt.bfloat16
FP8 = mybir.dt.float8e4

USE_FP8_KQ = True
X8_SCALE = 16.0
W8_SCALE = 64.0
KQ_FIX = 1.0 / (X8_SCALE * W8_SCALE)

_cached = {}


def build_program():
    nc = bacc.Bacc("TRN2", target_bir_lowering=False, debug=False,
                   enable_asserts=True, num_devices=NC)

    xt16_d = nc.dram_tensor("xt16", [TCH, P, KT, 512], BF16,
                            kind="ExternalInput").ap()
    if USE_FP8_KQ:
        xt8_d = nc.dram_tensor("xt8", [TCH, P, KT // 2, 2, 512], FP8,
                               kind="ExternalInput").ap()
        wqk_d = nc.dram_tensor("wqk", [P, 8, KT // 2, 2, P], FP8,
                               kind="ExternalInput").ap()
    else:
        wqk_d = nc.dram_tensor("wqk", [P, KT, 2 * JC], BF16,
                               kind="ExternalInput").ap()
    wv_d = nc.dram_tensor("wv", [P, KT, JC], BF16, kind="ExternalInput").ap()
    wo_d = nc.dram_tensor("wo", [P, JC // P, D], BF16,
                          kind="ExternalInput").ap()
    bqk_d = nc.dram_tensor("bqk", [P, 8], F32, kind="ExternalInput").ap()
    bvb_d = nc.dram_tensor("bvb", [P, JC], BF16, kind="ExternalInput").ap()
    out_d = nc.dram_tensor("out", [T, D], BF16, kind="ExternalOutput").ap()

    EXP = mybir.ActivationFunctionType.Exp
    COPY = mybir.ActivationFunctionType.Copy
    DR = mybir.MatmulPerfMode.DoubleRow
    SC = 0.125  # 1/sqrt(HD)

    with tile.TileContext(nc) as tc:
        with ExitStack() as ctx:
            persist = ctx.enter_context(tc.tile_pool(name="persist", bufs=1))
            xt16_sb = persist.tile([P, TCH, KT, 512], BF16, tag="xt16")
            if USE_FP8_KQ:
                xt8_sb = persist.tile([P, TCH, KT // 2, 2, 512], FP8,
                                      tag="xt8")
                wqk_sb = persist.tile([P, 8, KT // 2, 2, P], FP8, tag="wqk")
            else:
                wqk_sb = persist.tile([P, KT, 2 * JC], BF16, tag="wqk")
            wv_sb = persist.tile([P, KT, JC], BF16, tag="wv")
            wo_sb = persist.tile([P, JC // P, D], BF16, tag="wo")
            bqk_sb = persist.tile([P, 8], F32, tag="bqk")
            bvb_sb = persist.tile([P, JC], BF16, tag="bvb")
            qk_sb = persist.tile([P, 8, T], BF16, tag="qk")
            # [t, 8 x [v(64)|1]] + 64 pad cols so the AV stationary operand
            # can be sliced 128 wide
            VW = HPC * (HD + 1)
            vaug_f = persist.tile([P, TT, VW + HD], BF16, tag="vaug")
            ot_sb = persist.tile([P, JC // P, T], BF16, tag="ot")

            # ---- input DMAs (program order = queue order) ----
            if USE_FP8_KQ:
                for tci in range(2):
                    nc.sync.dma_start(xt8_sb[:, tci], xt8_d[tci])
                nc.sync.dma_start(xt16_sb[:, 0], xt16_d[0])
                for tci in range(2, TCH):
                    nc.sync.dma_start(xt8_sb[:, tci], xt8_d[tci])
                for tci in range(1, TCH):
                    nc.sync.dma_start(xt16_sb[:, tci], xt16_d[tci])
            else:
                for tci in range(TCH):
                    nc.sync.dma_start(xt16_sb[:, tci], xt16_d[tci])
            nc.gpsimd.dma_start(wqk_sb[:], wqk_d[:])
            nc.gpsimd.dma_start(wv_sb[:], wv_d[:])
            nc.gpsimd.dma_start(bqk_sb[:], bqk_d[:])
            nc.gpsimd.dma_start(bvb_sb[:], bvb_d[:])
            nc.gpsimd.dma_start(wo_sb[:], wo_d[:])

            ones64 = persist.tile([P, HD], F32, tag="ones64")
            nc.gpsimd.memset(ones64[:], 1.0)
            vaug = vaug_f[:, :, 0:VW].rearrange(
                "p t (h e) -> p t h e", h=HPC)          # [128, 16, 8, 65]
            for tt in range(TT):
                nc.gpsimd.memset(vaug[:, tt, :, HD:HD + 1], 1.0)
                nc.gpsimd.memset(vaug_f[:, tt, VW:VW + HD], 0.0)

            wtpool = ctx.enter_context(tc.tile_pool(name="wtpool", bufs=3))
            qpool = ctx.enter_context(tc.tile_pool(name="qpool", bufs=2))
            nrmpool = ctx.enter_context(tc.tile_pool(name="nrmpool", bufs=2))
            rcpool = ctx.enter_context(tc.tile_pool(name="rcpool", bufs=2))
            rbpool = ctx.enter_context(tc.tile_pool(name="rbpool", bufs=2))
            ostpool = ctx.enter_context(tc.tile_pool(name="ostpool", bufs=3))
            rdpool = ctx.enter_context(
                tc.tile_pool(name="rdpool", bufs=2, space="DRAM"))
            pss = ctx.enter_context(
                tc.tile_pool(name="pss", bufs=2, space="PSUM"))
            avp = ctx.enter_context(
                tc.tile_pool(name="avp", bufs=1, space="PSUM"))
            psf = ctx.enter_context(
                tc.tile_pool(name="psf", bufs=2, space="PSUM"))

            qpads = [qpool.tile([P, T], BF16, tag="qpad",
                                name=f"qpad_{i}") for i in range(2)]
            for i in range(2):
                nc.gpsimd.memset(qpads[i][:], 0.0)

            # ---------------- filler group builders ----------------
            # Each group is a list of closures; each closure emits one PE
            # matmul (the last also emits the psum evacuation on DVE).

            def kq_group(jcol, tci):
                """qk_sb j-tile jcol (0-3 = q j, 4-7 = kT j) over t-chunk."""
                wcol = jcol * P if jcol < 4 else JC + (jcol - 4) * P
                tsl = slice(tci * 512, (tci + 1) * 512)
                steps = []
                box = {}
                nsteps = KT // 2 if USE_FP8_KQ else KT

                def mk(i):
                    first, last = i == 0, i == nsteps - 1

                    def step():
                        if first:
                            box["ps"] = psf.tile([P, 512], F32, tag="psf",
                                                 name=f"kq_{jcol}_{tci}")
                        if USE_FP8_KQ:
                            nc.tensor.matmul(
                                box["ps"][:],
                                wqk_sb[:, jcol, i],
                                xt8_sb[:, tci, i],
                                start=first, stop=last, perf_mode=DR)
                        else:
                            nc.tensor.matmul(
                                box["ps"][:],
                                wqk_sb[:, i, wcol:wcol + P],
                                xt16_sb[:, tci, i],
                                start=first, stop=last)
                        if last:
                            if USE_FP8_KQ:
                                nc.vector.tensor_scalar(
                                    qk_sb[:, jcol, tsl], box["ps"][:],
                                    KQ_FIX, bqk_sb[:, jcol:jcol + 1],
                                    op0=mybir.AluOpType.mult,
                                    op1=mybir.AluOpType.add)
                            else:
                                nc.vector.tensor_scalar(
                                    qk_sb[:, jcol, tsl], box["ps"][:],
                                    bqk_sb[:, jcol:jcol + 1], None,
                                    op0=mybir.AluOpType.add)
                    return step
                for i in range(nsteps):
                    steps.append(mk(i))
                return steps

            def v_group(tglob):
                tci, tt = tglob // 4, tglob % 4
                steps = []
                box = {}

                def mk(k):
                    first, last = k == 0, k == KT - 1

                    def step():
                        if first:
                            box["ps"] = psf.tile([P, 512], F32, tag="psf",
                                                 name=f"v_{tglob}")
                        nc.tensor.matmul(
                            box["ps"][:],
                            xt16_sb[:, tci, k, tt * P:(tt + 1) * P],
                            wv_sb[:, k, :],
                            start=first, stop=last)
                        if last:
                            nc.vector.tensor_tensor(
                                vaug[:, tglob, :, 0:HD],
                                box["ps"][:].rearrange(
                                    "p (h d) -> p h d", h=HPC),
                                bvb_sb[:].rearrange("p (h d) -> p h d", h=HPC),
                                op=mybir.AluOpType.add)
                    return step
                for k in range(KT):
                    steps.append(mk(k))
                return steps

            ost_box = {}

            def outproj_group(tt, cc):
                steps = []
                box = {}

                def mk(jt):
                    first, last = jt == 0, jt == JC // P - 1

                    def step():
                        if first:
                            box["ps"] = psf.tile([P, 512], F32, tag="psf",
                                                 name=f"op_{tt}_{cc}")
                            if cc == 0:
                                ost_box[tt] = ostpool.tile(
                                    [P, D], BF16, tag="ost", name=f"ost_{tt}")
                        nc.tensor.matmul(
                            box["ps"][:],
                            ot_sb[:, jt, tt * P:(tt + 1) * P],
                            wo_sb[:, jt, cc * 512:(cc + 1) * 512],
                            start=first, stop=last)
                        if last:
                            if cc == 0 and tt >= 8:
                                nc.scalar.activation(
                                    ost_box[tt][:, 0:512], box["ps"][:],
                                    COPY)
                            else:
                                nc.vector.tensor_copy(
                                    ost_box[tt][:, cc * 512:(cc + 1) * 512],
                                    box["ps"][:])
                            if cc == 1:
                                eng = nc.sync if tt % 2 == 0 else nc.gpsimd
                                eng.dma_start(out_d[tt * P:(tt + 1) * P, :],
                                              ost_box[tt][:])
                    return step
                for jt in range(JC // P):
                    steps.append(mk(jt))
                return steps

            # ---------------- preamble ----------------
            for tci in range(TCH):
                for s in kq_group(4, tci):   # kT j0 tile (heads 0,1 k)
                    s()
                for s in kq_group(0, tci):   # q j0 tile (heads 0,1 q)
                    s()
            for tg in range(4):
                for s in v_group(tg):
                    s()
            nc.vector.tensor_copy(qpads[0][0:HD, :], qk_sb[0:HD, 0, :])

            # ---------------- filler window assignments ----------------
            # win key = (h, half); each window spans 16 attention iters.
            win_steps = {}
            win_start = {}
            win_len = {}

            def assign(windows, groups, start=0, length=16):
                flat = [s for g in groups for s in g]
                n = len(windows)
                for i, w in enumerate(windows):
                    win_steps[w] = flat[len(flat) * i // n:
                                        len(flat) * (i + 1) // n]
                    win_start[w] = start
                    win_len[w] = length

            # v tiles 4..15 must land just ahead of AV(h0, half0, k)
            assign([(0, 0)], [v_group(tg) for tg in range(4, 16)], length=14)
            assign([(0, 1), (1, 0)],
                   [kq_group(5, tci) for tci in range(TCH)]
                   + [kq_group(1, tci) for tci in range(TCH)])
            assign([(1, 1), (2, 0)],
                   [kq_group(6, tci) for tci in range(TCH)]
                   + [kq_group(2, tci) for tci in range(TCH)])
            assign([(2, 1), (3, 0), (3, 1), (4, 0)],
                   [kq_group(7, tci) for tci in range(TCH)]
                   + [kq_group(3, tci) for tci in range(TCH)])
            assign([(7, 1)],
                   [outproj_group(tt, cc) for tt in range(8)
                    for cc in range(2)], start=4, length=12)

            # ---------------- attention stream ----------------
            # Software pipeline: scores run one iteration ahead of exp so
            # the scalar engine never waits on the PE; AV trails by one
            # iteration; the previous half's psum evacuation is queued on
            # the scalar engine before the current exp so the AV psum bank
            # frees quickly.
            iters = [(h, half, k)
                     for h in range(HPC) for half in range(2)
                     for k in range(TT)]
            pss_tiles = {}
            af_tiles = {}
            sums_tiles = {}
            wt_tiles = {}

            def emit_scores(h, half, k):
                ps = pss.tile([P, 2, 512], F32, tag="pss",
                              name=f"s_{h}_{half}_{k}")
                kT2 = qk_sb[:, 4 + h // 2, :]
                qpad = qpads[h % 2]
                for c in range(2):
                    nc.tensor.matmul(
                        ps[:, c, :],
                        kT2[:, k * P:(k + 1) * P],
                        qpad[:, half * 1024 + c * 512:
                             half * 1024 + (c + 1) * 512],
                        start=True, stop=True)
                pss_tiles[(h, half, k)] = ps

            def emit_av(ph, phalf, pk):
                if pk == 0:
                    af_tiles[(ph, phalf)] = avp.tile(
                        [P, 2, 512], F32, tag="av", name=f"af_{ph}_{phalf}")
                paf = af_tiles[(ph, phalf)]
                pwt = wt_tiles.pop((ph, phalf, pk))
                for c in range(2):
                    nc.tensor.matmul(
                        paf[:, c, :],
                        vaug_f[:, pk, ph * (HD + 1):ph * (HD + 1) + P],
                        pwt[:, c * 512:(c + 1) * 512],
                        start=(pk == 0), stop=(pk == TT - 1))

            def emit_evac_finish(ph, phalf):
                paf = af_tiles.pop((ph, phalf))
                pb = (ph % 2) * 64
                jt = ph // 2
                hsl = slice(phalf * 1024, (phalf + 1) * 1024)
                last_head = ph == HPC - 1
                if phalf == 0 or last_head:
                    sums_tiles[ph] = nrmpool.tile([P, 512], F32, tag="sums",
                                                  name=f"sums_{ph}_{phalf}")
                    nc.gpsimd.memset(sums_tiles[ph][:], 1.0)
                sums = sums_tiles[ph]
                # o rows on DVE; denominator row via scalar engine (copy
                # shares the exp act table -> no table reload)
                nc.vector.tensor_copy(
                    ot_sb[pb:pb + HD, jt, hsl],
                    paf[0:HD].rearrange("p a b -> p (a b)"))
                for c in range(2):
                    cglob = c if last_head else 2 * phalf + c
                    nc.scalar.activation(
                        sums[32 * cglob:32 * cglob + 1, :],
                        paf[HD:HD + 1, c, :], COPY)
                # chunk sums live at 32-aligned partitions (engine partition
                # bases must be 32-aligned); reciprocal runs on the full tile
                # and downstream readers pick out the valid rows.
                rcp = rcpool.tile([P, 512], F32, tag="rcp",
                                  name=f"rcp_{ph}_{phalf}")
                nc.vector.reciprocal_approx_fast(rcp[:], sums[:])
                if last_head:
                    # last head: broadcast 1/den through the PE (fp32 rank-1
                    # matmuls) to skip the DRAM-bounce latency on the
                    # critical tail
                    for c in range(2):
                        rbp = psf.tile([P, 512], F32, tag="psf",
                                       name=f"rbp_{phalf}_{c}")
                        nc.tensor.matmul(
                            rbp[pb:pb + 64, :],
                            ones64[32 * c:32 * c + 1, :],
                            rcp[32 * c:32 * c + 1, :],
                            start=True, stop=True)
                        csl = slice(phalf * 1024 + c * 512,
                                    phalf * 1024 + (c + 1) * 512)
                        nc.vector.tensor_tensor(
                            ot_sb[pb:pb + 64, jt, csl],
                            ot_sb[pb:pb + 64, jt, csl],
                            rbp[pb:pb + 64, :], op=mybir.AluOpType.mult)
                else:
                    pbase = 64 * phalf
                    rd = rdpool.tile([2, 512], F32, tag="rd",
                                     name=f"rd_{ph}_{phalf}")
                    nc.sync.dma_start(rd[:], rcp[pbase:pbase + 33:32, :])
                    rb = rbpool.tile([P, 1024], F32, tag="rb",
                                     name=f"rb_{ph}_{phalf}")
                    rd_bcast = bass.AP(
                        tensor=rd.tensor, offset=rd.offset,
                        ap=[[0, 64], [512, 2], [1, 512]])
                    nc.sync.dma_start(
                        rb[pb:pb + 64, :].rearrange("p (c r) -> p c r", c=2),
                        rd_bcast)
                    nc.vector.tensor_mul(
                        ot_sb[pb:pb + 64, jt, hsl],
                        ot_sb[pb:pb + 64, jt, hsl],
                        rb[pb:pb + 64, :])

            win_emitted = {w: 0 for w in win_steps}
            prev = None
            emit_scores(*iters[0])
            for gi, (h, half, k) in enumerate(iters):
                if gi + 1 < len(iters):
                    emit_scores(*iters[gi + 1])
                w = (h, half)
                steps = win_steps.get(w, [])
                if steps:
                    s0 = win_start.get(w, 0)
                    slen = win_len.get(w, 16)
                    if k >= s0:
                        tgt = len(steps) if k == TT - 1 else min(
                            len(steps),
                            -(-len(steps) * (k - s0 + 1) // slen))
                        while win_emitted[w] < tgt:
                            steps[win_emitted[w]]()
                            win_emitted[w] += 1
                if prev is not None and prev[2] == TT - 1:
                    emit_av(*prev)
                    emit_evac_finish(prev[0], prev[1])
                wt = wtpool.tile([P, 1024], BF16, tag="wt",
                                 name=f"wt_{h}_{half}_{k}")
                nc.scalar.activation(
                    wt[:], pss_tiles.pop((h, half, k))[:].rearrange(
                        "p a b -> p (a b)"),
                    EXP, bias=0.0, scale=SC)
                wt_tiles[(h, half, k)] = wt
                if prev is not None and prev[2] != TT - 1:
                    emit_av(*prev)
                prev = (h, half, k)
                # prefetch next head's qpad late in this head
                if half == 1 and k == 12 and h + 1 < HPC:
                    nh = h + 1
                    npb = (nh % 2) * 64
                    nc.vector.tensor_copy(
                        qpads[nh % 2][npb:npb + HD, :],
                        qk_sb[npb:npb + HD, nh // 2, :])

            # drain: last AV + finish + outproj tiles 8..15
            emit_av(*prev)
            emit_evac_finish(prev[0], prev[1])
            for tt in range(8, 16):
                for cc in range(2):
                    for s in outproj_group(tt, cc):
                        s()

    nc.compile()
    return nc


def _prep_core_inputs(x, qkv_w, qkv_b, out_w, core):
    b, g = core // 2, core % 2
    jsl = slice(g * JC, (g + 1) * JC)

    xT = np.ascontiguousarray(x[b].T)                       # [1024, 2048]
    xk = xT.reshape(KT, P, TCH, 512)
    xt16 = np.ascontiguousarray(xk.transpose(2, 1, 0, 3))   # [4,128,8,512]

    Wq = qkv_w[0 * D:1 * D][jsl]                            # [512, 1024]
    Wk = qkv_w[1 * D:2 * D][jsl]
    Wv = qkv_w[2 * D:3 * D][jsl]
    WqkT = np.concatenate([Wq, Wk], axis=0).T               # [1024, 1024]
    WvT = Wv.T                                              # [1024, 512]
    wv = np.ascontiguousarray(
        WvT.reshape(KT, P, JC).transpose(1, 0, 2))          # [128, 8, 512]

    bq = qkv_b[0 * D:1 * D][jsl]
    bk = qkv_b[1 * D:2 * D][jsl]
    bv = qkv_b[2 * D:3 * D][jsl]
    bqk = np.ascontiguousarray(
        np.concatenate([bq, bk]).reshape(8, P).T)           # [128, 8]
    bvb = np.ascontiguousarray(np.tile(bv[None, :], (P, 1)))

    WoT = np.ascontiguousarray(out_w[:, jsl].T)             # [512, 1024]
    wo = np.ascontiguousarray(
        WoT.reshape(JC // P, P, D).transpose(1, 0, 2))      # [128, 4, 1024]

    inputs = {
        "xt16": xt16.astype(BF16_NP),
        "wv": wv.astype(BF16_NP),
        "wo": wo.astype(BF16_NP),
        "bqk": bqk.astype(np.float32),
        "bvb": bvb.astype(BF16_NP),
    }
    if USE_FP8_KQ:
        xt8 = np.ascontiguousarray(
            (xk * X8_SCALE).reshape(KT // 2, 2, P, TCH, 512)
            .transpose(3, 2, 0, 1, 4))                      # [4,128,4,2,512]
        wqk8 = np.ascontiguousarray(
            (WqkT * W8_SCALE).reshape(KT // 2, 2, P, 8, P)
            .transpose(2, 3, 0, 1, 4))                      # [128,8,4,2,128]
        inputs["xt8"] = xt8.astype(FP8_NP)
        inputs["wqk"] = wqk8.astype(FP8_NP)
    else:
        wqk = np.ascontiguousarray(
            WqkT.reshape(KT, P, 2 * JC).transpose(1, 0, 2))  # [128, 8, 1024]
        inputs["wqk"] = wqk.astype(BF16_NP)
    return inputs


def run(x, qkv_w, qkv_b, out_w, out_b, trace=False, tmpdir=None):
    if "nc" not in _cached:
        _cached["nc"] = build_program()
    nc = _cached["nc"]
    in_maps = [_prep_core_inputs(x, qkv_w, qkv_b, out_w, c) for c in range(NC)]
    res = run_bass_kernel_spmd(nc, in_maps, core_ids=list(range(NC)),
                               trace=trace, tmpdir=tmpdir)
    parts = np.stack([np.asarray(res.results[c]["out"], dtype=np.float32)
                      for c in range(NC)])                  # [8, T, D]
    out = parts.reshape(B, 2, T, D).sum(axis=1) + out_b[None, None, :]
    return out.astype(np.float32), res


def kernel(x, qkv_w, qkv_b, out_w, out_b):
    x = np.asarray(x, dtype=np.float32)
    qkv_w = np.asarray(qkv_w, dtype=np.float32)
    qkv_b = np.asarray(qkv_b, dtype=np.float32)
    out_w = np.asarray(out_w, dtype=np.float32)
    out_b = np.asarray(out_b, dtype=np.float32)
    out, _ = run(x, qkv_w, qkv_b, out_w, out_b, trace=False)
    return out


if __name__ == "__main__":
    import jax
    import reference
    inputs = {k: np.asarray(v) for k, v in reference.setup_inputs().items()}
    expected = np.asarray(reference.reference(**inputs))
    got = kernel(**inputs)
    err = np.linalg.norm(got - expected) / np.linalg.norm(expected)
    print("Relative error:", err)


# revision 20
# speedup vs baseline: 1.1448x; 1.0083x over previous
"""Multi-head attention (B=4, T=2048, D=1024, H=16) on 8 Trainium2 cores.

Sharding: batch (4-way) x head-half (2-way) -> 8 cores.
Core c handles batch b = c//2 and heads g*8..g*8+8 where g = c%2.

v2: single fully-pipelined instruction stream.
  - Attention (scores -> exp -> AV) starts at ~12us; the k/q/v projection
    GEMMs that are not needed immediately run as "filler" matmuls woven
    into the attention stream so the PE never idles while the scalar
    engine (exp) is the per-iteration rate limiter.
  - q/k projections optionally run as fp8e4 DoubleRow matmuls (2 k-tiles
    per instruction, 2x effective contraction rate). Inputs are scaled
    (x*16, W*64) into the fp8 normal range; the 1/1024 fixup is folded
    into the psum->SBUF evacuation. v stays bf16 (its quantization error
    would land directly on the output).
  - q/k/v biases are folded into the psum evacuation (DVE tensor_scalar /
    tensor_tensor) instead of rank-1 matmuls.
  - AV accumulates per (head, half-of-T) in a single 2-bank psum tile;
    scores psum 2x2 banks; 2 banks left for the filler GEMMs. Softmax
    denominators (ones-column of the augmented v) are evacuated by the
    scalar engine (copy shares the act table with exp), o rows by DVE.
  - Normalization per (head, half): reciprocal_approx_fast + DRAM-bounce
    partition broadcast + one DVE multiply. Out-projection tiles 0-7 run
    inside the last head's second-half window; tiles 8-15 after it.
  - Output is bf16 (halves DMA); host sums the two head-half partials.

Host: transposes/reshapes inputs per core (bf16/fp8), sums partials,
adds out_b.
"""

import numpy as np
import ml_dtypes
from contextlib import ExitStack

import concourse.bass as bass
import concourse.tile as tile
from concourse import bacc, mybir
from concourse.bass_utils import run_bass_kernel_spmd

BF16_NP = ml_dtypes.bfloat16
FP8_NP = ml_dtypes.float8_e4m3

B, T, D = 4, 2048, 1024
H, HD = 16, 64
P = 128
NC = 8
HPC = 8          # heads per core
JC = HPC * HD    # 512 head-dim columns per core
KT = D // P      # 8 contraction tiles for QKV
TT = T // P      # 16 t tiles
TCH = T // 512   # 4 t chunks of 512
F32 = mybir.dt.float32
BF16 = mybir.dt.bfloat16
FP8 = mybir.dt.float8e4

USE_FP8_KQ = True
X8_SCALE = 16.0
W8_SCALE = 64.0
KQ_FIX = 1.0 / (X8_SCALE * W8_SCALE)

_cached = {}


def build_program():
    nc = bacc.Bacc("TRN2", target_bir_lowering=False, debug=False,
                   enable_asserts=True, num_devices=NC)

    xt16_d = nc.dram_tensor("xt16", [TCH, P, KT, 512], BF16,
                            kind="ExternalInput").ap()
    if USE_FP8_KQ:
        xt8_d = nc.dram_tensor("xt8", [TCH, P, KT // 2, 2, 512], FP8,
                               kind="ExternalInput").ap()
        wqk_d = nc.dram_tensor("wqk", [P, 8, KT // 2, 2, P], FP8,
                               kind="ExternalInput").ap()
    else:
        wqk_d = nc.dram_tensor("wqk", [P, KT, 2 * JC], BF16,
                               kind="ExternalInput").ap()
    wv_d = nc.dram_tensor("wv", [P, KT, JC], BF16, kind="ExternalInput").ap()
    wo_d = nc.dram_tensor("wo", [P, JC // P, D], BF16,
                          kind="ExternalInput").ap()
    bqk_d = nc.dram_tensor("bqk", [P, 8], F32, kind="ExternalInput").ap()
    bvb_d = nc.dram_tensor("bvb", [P, JC], BF16, kind="ExternalInput").ap()
    out_d = nc.dram_tensor("out", [T, D], BF16, kind="ExternalOutput").ap()

    EXP = mybir.ActivationFunctionType.Exp
    COPY = mybir.ActivationFunctionType.Copy
    DR = mybir.MatmulPerfMode.DoubleRow
    SC = 0.125  # 1/sqrt(HD)

    with tile.TileContext(nc) as tc:
        with ExitStack() as ctx:
            persist = ctx.enter_context(tc.tile_pool(name="persist", bufs=1))
            xt16_sb = persist.tile([P, TCH, KT, 512], BF16, tag="xt16")
            if USE_FP8_KQ:
                xt8_sb = persist.tile([P, TCH, KT // 2, 2, 512], FP8,
                                      tag="xt8")
                wqk_sb = persist.tile([P, 8, KT // 2, 2, P], FP8, tag="wqk")
            else:
                wqk_sb = persist.tile([P, KT, 2 * JC], BF16, tag="wqk")
            wv_sb = persist.tile([P, KT, JC], BF16, tag="wv")
            wo_sb = persist.tile([P, JC // P, D], BF16, tag="wo")
            bqk_sb = persist.tile([P, 8], F32, tag="bqk")
            bvb_sb = persist.tile([P, JC], BF16, tag="bvb")
            qk_sb = persist.tile([P, 8, T], BF16, tag="qk")
            # [t, 8 x [v(64)|1]] + 64 pad cols so the AV stationary operand
            # can be sliced 128 wide
            VW = HPC * (HD + 1)
            vaug_f = persist.tile([P, TT, VW + HD], BF16, tag="vaug")
            ot_sb = persist.tile([P, JC // P, T], BF16, tag="ot")
            # out-projection partial sums over head-pairs 0..2 (bf16), so
            # most of the out-proj runs in the filler-less late windows
            ost_part = persist.tile([P, TT, D], BF16, tag="ostp")

            # ---- input DMAs (program order = queue order) ----
            if USE_FP8_KQ:
                # fine-grained first transfers so the first matmul can
                # start as soon as ~384KB have landed
                for kp in range(KT // 2):
                    nc.sync.dma_start(xt8_sb[:, 0, kp], xt8_d[0, :, kp])
                nc.sync.dma_start(xt8_sb[:, 1], xt8_d[1])
                nc.sync.dma_start(xt16_sb[:, 0], xt16_d[0])
                for tci in range(2, TCH):
                    nc.sync.dma_start(xt8_sb[:, tci], xt8_d[tci])
                for tci in range(1, TCH):
                    nc.sync.dma_start(xt16_sb[:, tci], xt16_d[tci])
                for kp in range(KT // 2):
                    nc.gpsimd.dma_start(wqk_sb[:, :, kp], wqk_d[:, :, kp])
            else:
                for tci in range(TCH):
                    nc.sync.dma_start(xt16_sb[:, tci], xt16_d[tci])
                nc.gpsimd.dma_start(wqk_sb[:], wqk_d[:])
            nc.gpsimd.dma_start(wv_sb[:], wv_d[:])
            nc.gpsimd.dma_start(bqk_sb[:], bqk_d[:])
            nc.gpsimd.dma_start(bvb_sb[:], bvb_d[:])
            nc.gpsimd.dma_start(wo_sb[:], wo_d[:])

            ones64 = persist.tile([P, HD], F32, tag="ones64")
            nc.gpsimd.memset(ones64[:], 1.0)
            vaug = vaug_f[:, :, 0:VW].rearrange(
                "p t (h e) -> p t h e", h=HPC)          # [128, 16, 8, 65]
            for tt in range(TT):
                nc.gpsimd.memset(vaug[:, tt, :, HD:HD + 1], 1.0)
                nc.gpsimd.memset(vaug_f[:, tt, VW:VW + HD], 0.0)

            wtpool = ctx.enter_context(tc.tile_pool(name="wtpool", bufs=3))
            qpool = ctx.enter_context(tc.tile_pool(name="qpool", bufs=2))
            nrmpool = ctx.enter_context(tc.tile_pool(name="nrmpool", bufs=2))
            rcpool = ctx.enter_context(tc.tile_pool(name="rcpool", bufs=1))
            rbpool = ctx.enter_context(tc.tile_pool(name="rbpool", bufs=1))
            ostpool = ctx.enter_context(tc.tile_pool(name="ostpool", bufs=2))
            rdpool = ctx.enter_context(
                tc.tile_pool(name="rdpool", bufs=2, space="DRAM"))
            pss = ctx.enter_context(
                tc.tile_pool(name="pss", bufs=2, space="PSUM"))
            avp = ctx.enter_context(
                tc.tile_pool(name="avp", bufs=1, space="PSUM"))
            psf = ctx.enter_context(
                tc.tile_pool(name="psf", bufs=2, space="PSUM"))

            qpads = [qpool.tile([P, T], BF16, tag="qpad",
                                name=f"qpad_{i}") for i in range(2)]
            for i in range(2):
                nc.gpsimd.memset(qpads[i][:], 0.0)

            # ---------------- filler group builders ----------------
            # Each group is a list of closures; each closure emits one PE
            # matmul (the last also emits the psum evacuation on DVE).

            def kq_group(jcol, tci):
                """qk_sb j-tile jcol (0-3 = q j, 4-7 = kT j) over t-chunk."""
                wcol = jcol * P if jcol < 4 else JC + (jcol - 4) * P
                tsl = slice(tci * 512, (tci + 1) * 512)
                steps = []
                box = {}
                nsteps = KT // 2 if USE_FP8_KQ else KT

                def mk(i):
                    first, last = i == 0, i == nsteps - 1

                    def step():
                        if first:
                            box["ps"] = psf.tile([P, 512], F32, tag="psf",
                                                 name=f"kq_{jcol}_{tci}")
                        if USE_FP8_KQ:
                            nc.tensor.matmul(
                                box["ps"][:],
                                wqk_sb[:, jcol, i],
                                xt8_sb[:, tci, i],
                                start=first, stop=last, perf_mode=DR)
                        else:
                            nc.tensor.matmul(
                                box["ps"][:],
                                wqk_sb[:, i, wcol:wcol + P],
                                xt16_sb[:, tci, i],
                                start=first, stop=last)
                        if last:
                            if USE_FP8_KQ:
                                nc.vector.tensor_scalar(
                                    qk_sb[:, jcol, tsl], box["ps"][:],
                                    KQ_FIX, bqk_sb[:, jcol:jcol + 1],
                                    op0=mybir.AluOpType.mult,
                                    op1=mybir.AluOpType.add)
                            else:
                                nc.vector.tensor_scalar(
                                    qk_sb[:, jcol, tsl], box["ps"][:],
                                    bqk_sb[:, jcol:jcol + 1], None,
                                    op0=mybir.AluOpType.add)
                    return step
                for i in range(nsteps):
                    steps.append(mk(i))
                return steps

            def v_group(tglob):
                tci, tt = tglob // 4, tglob % 4
                steps = []
                box = {}

                def mk(k):
                    first, last = k == 0, k == KT - 1

                    def step():
                        if first:
                            box["ps"] = psf.tile([P, 512], F32, tag="psf",
                                                 name=f"v_{tglob}")
                        nc.tensor.matmul(
                            box["ps"][:],
                            xt16_sb[:, tci, k, tt * P:(tt + 1) * P],
                            wv_sb[:, k, :],
                            start=first, stop=last)
                        if last:
                            nc.vector.tensor_tensor(
                                vaug[:, tglob, :, 0:HD],
                                box["ps"][:].rearrange(
                                    "p (h d) -> p h d", h=HPC),
                                bvb_sb[:].rearrange("p (h d) -> p h d", h=HPC),
                                op=mybir.AluOpType.add)
                    return step
                for k in range(KT):
                    steps.append(mk(k))
                return steps

            ost_box = {}

            def outproj_part_group(tt, cc):
                """jt 0..2 partial accumulation (needs heads 0..5 only)."""
                steps = []
                box = {}

                def mk(jt):
                    first, last = jt == 0, jt == 2

                    def step():
                        if first:
                            box["ps"] = psf.tile([P, 512], F32, tag="psf",
                                                 name=f"opp_{tt}_{cc}")
                        nc.tensor.matmul(
                            box["ps"][:],
                            ot_sb[:, jt, tt * P:(tt + 1) * P],
                            wo_sb[:, jt, cc * 512:(cc + 1) * 512],
                            start=first, stop=last)
                        if last:
                            nc.vector.tensor_copy(
                                ost_part[:, tt, cc * 512:(cc + 1) * 512],
                                box["ps"][:])
                    return step
                for jt in range(3):
                    steps.append(mk(jt))
                return steps

            def outproj_final_group(tt, cc):
                """jt 3 matmul (heads 6,7) + add of the jt0-2 partial."""
                steps = []

                def step():
                    ps = psf.tile([P, 512], F32, tag="psf",
                                  name=f"opf_{tt}_{cc}")
                    if cc == 0:
                        ost_box[tt] = ostpool.tile(
                            [P, D], BF16, tag="ost", name=f"ost_{tt}")
                    nc.tensor.matmul(
                        ps[:],
                        ot_sb[:, 3, tt * P:(tt + 1) * P],
                        wo_sb[:, 3, cc * 512:(cc + 1) * 512],
                        start=True, stop=True)
                    nc.vector.tensor_tensor(
                        ost_box[tt][:, cc * 512:(cc + 1) * 512],
                        ps[:],
                        ost_part[:, tt, cc * 512:(cc + 1) * 512],
                        op=mybir.AluOpType.add)
                    if cc == 1:
                        eng = nc.sync if tt % 2 == 0 else nc.gpsimd
                        eng.dma_start(out_d[tt * P:(tt + 1) * P, :],
                                      ost_box[tt][:])
                steps.append(step)
                return steps

            # ---------------- preamble ----------------
            for tci in range(TCH):
                for s in kq_group(4, tci):   # kT j0 tile (heads 0,1 k)
                    s()
                for s in kq_group(0, tci):   # q j0 tile (heads 0,1 q)
                    s()
            for tg in range(4):
                for s in v_group(tg):
                    s()
            nc.vector.tensor_copy(qpads[0][0:HD, :], qk_sb[0:HD, 0, :])

            # ---------------- filler window assignments ----------------
            # win key = (h, half); each window spans 16 attention iters.
            win_steps = {}
            win_start = {}
            win_len = {}

            def assign(windows, groups, start=0, length=16):
                flat = [s for g in groups for s in g]
                n = len(windows)
                for i, w in enumerate(windows):
                    win_steps[w] = flat[len(flat) * i // n:
                                        len(flat) * (i + 1) // n]
                    win_start[w] = start
                    win_len[w] = length

            # v tiles 4..15 must land just ahead of AV(h0, half0, k)
            assign([(0, 0)], [v_group(tg) for tg in range(4, 16)], length=14)
            assign([(0, 1), (1, 0)],
                   [kq_group(5, tci) for tci in range(TCH)]
                   + [kq_group(1, tci) for tci in range(TCH)])
            assign([(1, 1), (2, 0)],
                   [kq_group(6, tci) for tci in range(TCH)]
                   + [kq_group(2, tci) for tci in range(TCH)])
            assign([(2, 1), (3, 0), (3, 1), (4, 0)],
                   [kq_group(7, tci) for tci in range(TCH)]
                   + [kq_group(3, tci) for tci in range(TCH)])
            # out-proj partials (jt 0-2) fill the otherwise-empty late
            # windows; tiles 0-7 (t columns of half 0) need heads 0-5 half0
            # (done by (5,1)), tiles 8-15 need half1 (done by (6,1)).
            assign([(5, 1)], [outproj_part_group(tt, cc)
                              for tt in range(0, 4) for cc in range(2)],
                   start=1)
            assign([(6, 0)], [outproj_part_group(tt, cc)
                              for tt in range(4, 8) for cc in range(2)])
            assign([(6, 1)], [outproj_part_group(tt, cc)
                              for tt in range(8, 12) for cc in range(2)],
                   start=1)
            assign([(7, 0)], [outproj_part_group(tt, cc)
                              for tt in range(12, 16) for cc in range(2)])
            assign([(7, 1)],
                   [outproj_final_group(tt, cc) for tt in range(8)
                    for cc in range(2)], start=3, length=13)

            # ---------------- attention stream ----------------
            # Software pipeline: scores run one iteration ahead of exp so
            # the scalar engine never waits on the PE; AV trails by one
            # iteration; the previous half's psum evacuation is queued on
            # the scalar engine before the current exp so the AV psum bank
            # frees quickly.
            iters = [(h, half, k)
                     for h in range(HPC) for half in range(2)
                     for k in range(TT)]
            pss_tiles = {}
            af_tiles = {}
            sums_tiles = {}
            wt_tiles = {}

            def emit_scores(h, half, k):
                ps = pss.tile([P, 2, 512], F32, tag="pss",
                              name=f"s_{h}_{half}_{k}")
                kT2 = qk_sb[:, 4 + h // 2, :]
                qpad = qpads[h % 2]
                for c in range(2):
                    nc.tensor.matmul(
                        ps[:, c, :],
                        kT2[:, k * P:(k + 1) * P],
                        qpad[:, half * 1024 + c * 512:
                             half * 1024 + (c + 1) * 512],
                        start=True, stop=True)
                pss_tiles[(h, half, k)] = ps

            def emit_av(ph, phalf, pk):
                if pk == 0:
                    af_tiles[(ph, phalf)] = avp.tile(
                        [P, 2, 512], F32, tag="av", name=f"af_{ph}_{phalf}")
                paf = af_tiles[(ph, phalf)]
                pwt = wt_tiles.pop((ph, phalf, pk))
                for c in range(2):
                    nc.tensor.matmul(
                        paf[:, c, :],
                        vaug_f[:, pk, ph * (HD + 1):ph * (HD + 1) + P],
                        pwt[:, c * 512:(c + 1) * 512],
                        start=(pk == 0), stop=(pk == TT - 1))

            def emit_evac_finish(ph, phalf):
                paf = af_tiles.pop((ph, phalf))
                pb = (ph % 2) * 64
                jt = ph // 2
                hsl = slice(phalf * 1024, (phalf + 1) * 1024)
                if phalf == 0:
                    sums_tiles[ph] = nrmpool.tile([P, 1024], F32, tag="sums",
                                                  name=f"sums_{ph}")
                    nc.gpsimd.memset(sums_tiles[ph][0:64, :], 1.0)
                sums = sums_tiles[ph]
                spart = 32 * phalf
                # o rows on DVE; denominator row via scalar engine in one
                # [1,1024] copy (shares the exp act table -> no reload)
                nc.vector.tensor_copy(
                    ot_sb[pb:pb + HD, jt, hsl],
                    paf[0:HD].rearrange("p a b -> p (a b)"))
                nc.scalar.activation(
                    sums[spart:spart + 1, :],
                    paf[HD:HD + 1].rearrange("p a b -> p (a b)"), COPY)
                rcp = rcpool.tile([P, 1024], F32, tag="rcp",
                                  name=f"rcp_{ph}_{phalf}")
                nc.vector.reciprocal_approx_fast(
                    rcp[0:64, :], sums[0:64, :])
                if ph == HPC - 1:
                    # last head: broadcast 1/den through the PE (fp32 rank-1
                    # matmuls) to skip the DRAM-bounce latency on the
                    # critical tail
                    for c in range(2):
                        rbp = psf.tile([P, 512], F32, tag="psf",
                                       name=f"rbp_{phalf}_{c}")
                        nc.tensor.matmul(
                            rbp[pb:pb + 64, :],
                            ones64[spart:spart + 1, :],
                            rcp[spart:spart + 1, c * 512:(c + 1) * 512],
                            start=True, stop=True)
                        csl = slice(phalf * 1024 + c * 512,
                                    phalf * 1024 + (c + 1) * 512)
                        nc.vector.tensor_tensor(
                            ot_sb[pb:pb + 64, jt, csl],
                            ot_sb[pb:pb + 64, jt, csl],
                            rbp[pb:pb + 64, :], op=mybir.AluOpType.mult)
                else:
                    rd = rdpool.tile([2, 512], F32, tag="rd",
                                     name=f"rd_{ph}_{phalf}")
                    nc.sync.dma_start(
                        rd[:].rearrange("a b -> (a b)"),
                        rcp[spart:spart + 1, :])
                    rb = rbpool.tile([P, 1024], F32, tag="rb",
                                     name=f"rb_{ph}_{phalf}")
                    rd_bcast = bass.AP(
                        tensor=rd.tensor, offset=rd.offset,
                        ap=[[0, 64], [512, 2], [1, 512]])
                    nc.sync.dma_start(
                        rb[pb:pb + 64, :].rearrange("p (c r) -> p c r", c=2),
                        rd_bcast)
                    nc.vector.tensor_mul(
                        ot_sb[pb:pb + 64, jt, hsl],
                        ot_sb[pb:pb + 64, jt, hsl],
                        rb[pb:pb + 64, :])

            win_emitted = {w: 0 for w in win_steps}
            prev = None
            emit_scores(*iters[0])
            for gi, (h, half, k) in enumerate(iters):
                if gi + 1 < len(iters):
                    emit_scores(*iters[gi + 1])
                w = (h, half)
                steps = win_steps.get(w, [])
                if steps:
                    s0 = win_start.get(w, 0)
                    slen = win_len.get(w, 16)
                    if k >= s0:
                        tgt = len(steps) if k == TT - 1 else min(
                            len(steps),
                            -(-len(steps) * (k - s0 + 1) // slen)
                            + (3 if k == s0 else 0))
                        while win_emitted[w] < tgt:
                            steps[win_emitted[w]]()
                            win_emitted[w] += 1
                if prev is not None and prev[2] == TT - 1:
                    emit_av(*prev)
                    emit_evac_finish(prev[0], prev[1])
                wt = wtpool.tile([P, 1024], BF16, tag="wt",
                                 name=f"wt_{h}_{half}_{k}")
                nc.scalar.activation(
                    wt[:], pss_tiles.pop((h, half, k))[:].rearrange(
                        "p a b -> p (a b)"),
                    EXP, bias=0.0, scale=SC)
                wt_tiles[(h, half, k)] = wt
                if prev is not None and prev[2] != TT - 1:
                    emit_av(*prev)
                prev = (h, half, k)
                # prefetch next head's qpad late in this head
                if half == 1 and k == 12 and h + 1 < HPC:
                    nh = h + 1
                    npb = (nh % 2) * 64
                    nc.vector.tensor_copy(
                        qpads[nh % 2][npb:npb + HD, :],
                        qk_sb[npb:npb + HD, nh // 2, :])

            # drain: last AV + finish + outproj finals for tiles 8..15
            emit_av(*prev)
            emit_evac_finish(prev[0], prev[1])
            for tt in range(8, 16):
                for cc in range(2):
                    for s in outproj_final_group(tt, cc):
                        s()

    nc.compile()
    return nc


def _prep_core_inputs(x, qkv_w, qkv_b, out_w, core):
    b, g = core // 2, core % 2
    jsl = slice(g * JC, (g + 1) * JC)

    xT = np.ascontiguousarray(x[b].T)                       # [1024, 2048]
    xk = xT.reshape(KT, P, TCH, 512)
    xt16 = np.ascontiguousarray(xk.transpose(2, 1, 0, 3))   # [4,128,8,512]

    Wq = qkv_w[0 * D:1 * D][jsl]                            # [512, 1024]
    Wk = qkv_w[1 * D:2 * D][jsl]
    Wv = qkv_w[2 * D:3 * D][jsl]
    WqkT = np.concatenate([Wq, Wk], axis=0).T               # [1024, 1024]
    WvT = Wv.T                                              # [1024, 512]
    wv = np.ascontiguousarray(
        WvT.reshape(KT, P, JC).transpose(1, 0, 2))          # [128, 8, 512]

    bq = qkv_b[0 * D:1 * D][jsl]
    bk = qkv_b[1 * D:2 * D][jsl]
    bv = qkv_b[2 * D:3 * D][jsl]
    bqk = np.ascontiguousarray(
        np.concatenate([bq, bk]).reshape(8, P).T)           # [128, 8]
    bvb = np.ascontiguousarray(np.tile(bv[None, :], (P, 1)))

    WoT = np.ascontiguousarray(out_w[:, jsl].T)             # [512, 1024]
    wo = np.ascontiguousarray(
        WoT.reshape(JC // P, P, D).transpose(1, 0, 2))      # [128, 4, 1024]

    inputs = {
        "xt16": xt16.astype(BF16_NP),
        "wv": wv.astype(BF16_NP),
        "wo": wo.astype(BF16_NP),
        "bqk": bqk.astype(np.float32),
        "bvb": bvb.astype(BF16_NP),
    }
    if USE_FP8_KQ:
        xt8 = np.ascontiguousarray(
            (xk * X8_SCALE).reshape(KT // 2, 2, P, TCH, 512)
            .transpose(3, 2, 0, 1, 4))                      # [4,128,4,2,512]
        wqk8 = np.ascontiguousarray(
            (WqkT * W8_SCALE).reshape(KT // 2, 2, P, 8, P)
            .transpose(2, 3, 0, 1, 4))                      # [128,8,4,2,128]
        inputs["xt8"] = xt8.astype(FP8_NP)
        inputs["wqk"] = wqk8.astype(FP8_NP)
    else:
        wqk = np.ascontiguousarray(
            WqkT.reshape(KT, P, 2 * JC).transpose(1, 0, 2))  # [128, 8, 1024]
        inputs["wqk"] = wqk.astype(BF16_NP)
    return inputs


def run(x, qkv_w, qkv_b, out_w, out_b, trace=False, tmpdir=None):
    if "nc" not in _cached:
        _cached["nc"] = build_program()
    nc = _cached["nc"]
    in_maps = [_prep_core_inputs(x, qkv_w, qkv_b, out_w, c) for c in range(NC)]
    res = run_bass_kernel_spmd(nc, in_maps, core_ids=list(range(NC)),
                               trace=trace, tmpdir=tmpdir)
    parts = np.stack([np.asarray(res.results[c]["out"], dtype=np.float32)
                      for c in range(NC)])                  # [8, T, D]
    out = parts.reshape(B, 2, T, D).sum(axis=1) + out_b[None, None, :]
    return out.astype(np.float32), res


def kernel(x, qkv_w, qkv_b, out_w, out_b):
    x = np.asarray(x, dtype=np.float32)
    qkv_w = np.asarray(qkv_w, dtype=np.float32)
    qkv_b = np.asarray(qkv_b, dtype=np.float32)
    out_w = np.asarray(out_w, dtype=np.float32)
    out_b = np.asarray(out_b, dtype=np.float32)
    out, _ = run(x, qkv_w, qkv_b, out_w, out_b, trace=False)
    return out


if __name__ == "__main__":
    import jax
    import reference
    inputs = {k: np.asarray(v) for k, v in reference.setup_inputs().items()}
    expected = np.asarray(reference.reference(**inputs))
    got = kernel(**inputs)
    err = np.linalg.norm(got - expected) / np.linalg.norm(expected)
    print("Relative error:", err)


# revision 22
# speedup vs baseline: 1.1540x; 1.0080x over previous
"""Multi-head attention (B=4, T=2048, D=1024, H=16) on 8 Trainium2 cores.

Sharding: batch (4-way) x head-half (2-way) -> 8 cores.
Core c handles batch b = c//2 and heads g*8..g*8+8 where g = c%2.

v2: single fully-pipelined instruction stream.
  - Attention (scores -> exp -> AV) starts at ~12us; the k/q/v projection
    GEMMs that are not needed immediately run as "filler" matmuls woven
    into the attention stream so the PE never idles while the scalar
    engine (exp) is the per-iteration rate limiter.
  - q/k projections optionally run as fp8e4 DoubleRow matmuls (2 k-tiles
    per instruction, 2x effective contraction rate). Inputs are scaled
    (x*16, W*64) into the fp8 normal range; the 1/1024 fixup is folded
    into the psum->SBUF evacuation. v stays bf16 (its quantization error
    would land directly on the output).
  - q/k/v biases are folded into the psum evacuation (DVE tensor_scalar /
    tensor_tensor) instead of rank-1 matmuls.
  - AV accumulates per (head, half-of-T) in a single 2-bank psum tile;
    scores psum 2x2 banks; 2 banks left for the filler GEMMs. Softmax
    denominators (ones-column of the augmented v) are evacuated by the
    scalar engine (copy shares the act table with exp), o rows by DVE.
  - Normalization per (head, half): reciprocal_approx_fast + DRAM-bounce
    partition broadcast + one DVE multiply. Out-projection tiles 0-7 run
    inside the last head's second-half window; tiles 8-15 after it.
  - Output is bf16 (halves DMA); host sums the two head-half partials.

Host: transposes/reshapes inputs per core (bf16/fp8), sums partials,
adds out_b.
"""

import numpy as np
import ml_dtypes
from contextlib import ExitStack

import concourse.bass as bass
import concourse.tile as tile
from concourse import bacc, mybir
from concourse.bass_utils import run_bass_kernel_spmd

BF16_NP = ml_dtypes.bfloat16
FP8_NP = ml_dtypes.float8_e4m3

B, T, D = 4, 2048, 1024
H, HD = 16, 64
P = 128
NC = 8
HPC = 8          # heads per core
JC = HPC * HD    # 512 head-dim columns per core
KT = D // P      # 8 contraction tiles for QKV
TT = T // P      # 16 t tiles
TCH = T // 512   # 4 t chunks of 512
F32 = mybir.dt.float32
BF16 = mybir.dt.bfloat16
FP8 = mybir.dt.float8e4

USE_FP8_KQ = True
X8_SCALE = 16.0
W8_SCALE = 64.0
KQ_FIX = 1.0 / (X8_SCALE * W8_SCALE)

_cached = {}


def build_program():
    nc = bacc.Bacc("TRN2", target_bir_lowering=False, debug=False,
                   enable_asserts=True, num_devices=NC)

    xt16_d = nc.dram_tensor("xt16", [TCH, P, KT, 512], BF16,
                            kind="ExternalInput").ap()
    if USE_FP8_KQ:
        xt8_d = nc.dram_tensor("xt8", [TCH, P, KT // 2, 2, 512], FP8,
                               kind="ExternalInput").ap()
        wqk_d = nc.dram_tensor("wqk", [P, 8, KT // 2, 2, P], FP8,
                               kind="ExternalInput").ap()
    else:
        wqk_d = nc.dram_tensor("wqk", [P, KT, 2 * JC], BF16,
                               kind="ExternalInput").ap()
    wv_d = nc.dram_tensor("wv", [P, KT, JC], BF16, kind="ExternalInput").ap()
    wo_d = nc.dram_tensor("wo", [P, JC // P, D], BF16,
                          kind="ExternalInput").ap()
    bqk_d = nc.dram_tensor("bqk", [P, 8], F32, kind="ExternalInput").ap()
    ident_d = nc.dram_tensor("ident", [P, P], BF16, kind="ExternalInput").ap()
    bvb_d = nc.dram_tensor("bvb", [P, JC], BF16, kind="ExternalInput").ap()
    out_d = nc.dram_tensor("out", [T, D], BF16, kind="ExternalOutput").ap()

    EXP = mybir.ActivationFunctionType.Exp
    COPY = mybir.ActivationFunctionType.Copy
    DR = mybir.MatmulPerfMode.DoubleRow
    SC = 0.125  # 1/sqrt(HD)

    with tile.TileContext(nc) as tc:
        with ExitStack() as ctx:
            persist = ctx.enter_context(tc.tile_pool(name="persist", bufs=1))
            xt16_sb = persist.tile([P, TCH, KT, 512], BF16, tag="xt16")
            if USE_FP8_KQ:
                xt8_sb = persist.tile([P, TCH, KT // 2, 2, 512], FP8,
                                      tag="xt8")
                wqk_sb = persist.tile([P, 8, KT // 2, 2, P], FP8, tag="wqk")
            else:
                wqk_sb = persist.tile([P, KT, 2 * JC], BF16, tag="wqk")
            wv_sb = persist.tile([P, KT, JC], BF16, tag="wv")
            wo_sb = persist.tile([P, JC // P, D], BF16, tag="wo")
            bqk_sb = persist.tile([P, 8], F32, tag="bqk")
            ident_sb = persist.tile([P, P], BF16, tag="ident")
            bvb_sb = persist.tile([P, JC], BF16, tag="bvb")
            qk_sb = persist.tile([P, 8, T], BF16, tag="qk")
            # [t, 8 x [v(64)|1]] + 64 pad cols so the AV stationary operand
            # can be sliced 128 wide
            VW = HPC * (HD + 1)
            vaug_f = persist.tile([P, TT, VW + HD], BF16, tag="vaug")
            ot_sb = persist.tile([P, JC // P, T], BF16, tag="ot")
            # out-projection partial sums over head-pairs 0..2 (bf16), so
            # most of the out-proj runs in the filler-less late windows
            ost_part = persist.tile([P, TT, D], BF16, tag="ostp")

            # ---- input DMAs (program order = queue order) ----
            if USE_FP8_KQ:
                for tci in range(2):
                    nc.sync.dma_start(xt8_sb[:, tci], xt8_d[tci])
                nc.sync.dma_start(xt16_sb[:, 0], xt16_d[0])
                for tci in range(2, TCH):
                    nc.sync.dma_start(xt8_sb[:, tci], xt8_d[tci])
                for tci in range(1, TCH):
                    nc.sync.dma_start(xt16_sb[:, tci], xt16_d[tci])
                nc.gpsimd.dma_start(wqk_sb[:], wqk_d[:])
            else:
                for tci in range(TCH):
                    nc.sync.dma_start(xt16_sb[:, tci], xt16_d[tci])
                nc.gpsimd.dma_start(wqk_sb[:], wqk_d[:])
            nc.gpsimd.dma_start(wv_sb[:], wv_d[:])
            nc.gpsimd.dma_start(bqk_sb[:], bqk_d[:])
            nc.gpsimd.dma_start(ident_sb[:], ident_d[:])
            nc.gpsimd.dma_start(bvb_sb[:], bvb_d[:])
            nc.gpsimd.dma_start(wo_sb[:], wo_d[:])

            ones64 = persist.tile([P, HD], F32, tag="ones64")
            nc.gpsimd.memset(ones64[:], 1.0)
            vaug = vaug_f[:, :, 0:VW].rearrange(
                "p t (h e) -> p t h e", h=HPC)          # [128, 16, 8, 65]
            for tt in range(TT):
                nc.gpsimd.memset(vaug[:, tt, :, HD:HD + 1], 1.0)
                nc.gpsimd.memset(vaug_f[:, tt, VW:VW + HD], 0.0)

            wtpool = ctx.enter_context(tc.tile_pool(name="wtpool", bufs=3))
            qpool = ctx.enter_context(tc.tile_pool(name="qpool", bufs=2))
            nrmpool = ctx.enter_context(tc.tile_pool(name="nrmpool", bufs=2))
            rcpool = ctx.enter_context(tc.tile_pool(name="rcpool", bufs=1))
            rbpool = ctx.enter_context(tc.tile_pool(name="rbpool", bufs=1))
            ostpool = ctx.enter_context(tc.tile_pool(name="ostpool", bufs=2))
            rdpool = ctx.enter_context(
                tc.tile_pool(name="rdpool", bufs=2, space="DRAM"))
            pss = ctx.enter_context(
                tc.tile_pool(name="pss", bufs=2, space="PSUM"))
            avp = ctx.enter_context(
                tc.tile_pool(name="avp", bufs=1, space="PSUM"))
            psf = ctx.enter_context(
                tc.tile_pool(name="psf", bufs=2, space="PSUM"))

            qpads = [qpool.tile([P, T], BF16, tag="qpad",
                                name=f"qpad_{i}") for i in range(2)]
            for i in range(2):
                nc.gpsimd.memset(qpads[i][:], 0.0)

            # ---------------- filler group builders ----------------
            # Each group is a list of closures; each closure emits one PE
            # matmul (the last also emits the psum evacuation on DVE).

            def kq_group(jcol, tci):
                """qk_sb j-tile jcol (0-3 = q j, 4-7 = kT j) over t-chunk."""
                wcol = jcol * P if jcol < 4 else JC + (jcol - 4) * P
                tsl = slice(tci * 512, (tci + 1) * 512)
                steps = []
                box = {}
                nsteps = KT // 2 if USE_FP8_KQ else KT

                def mk(i):
                    first, last = i == 0, i == nsteps - 1

                    def step():
                        if first:
                            box["ps"] = psf.tile([P, 512], F32, tag="psf",
                                                 name=f"kq_{jcol}_{tci}")
                        if USE_FP8_KQ:
                            nc.tensor.matmul(
                                box["ps"][:],
                                wqk_sb[:, jcol, i],
                                xt8_sb[:, tci, i],
                                start=first, stop=last, perf_mode=DR)
                        else:
                            nc.tensor.matmul(
                                box["ps"][:],
                                wqk_sb[:, i, wcol:wcol + P],
                                xt16_sb[:, tci, i],
                                start=first, stop=last)
                        if last:
                            if USE_FP8_KQ:
                                nc.vector.tensor_scalar(
                                    qk_sb[:, jcol, tsl], box["ps"][:],
                                    KQ_FIX, bqk_sb[:, jcol:jcol + 1],
                                    op0=mybir.AluOpType.mult,
                                    op1=mybir.AluOpType.add)
                            else:
                                nc.vector.tensor_scalar(
                                    qk_sb[:, jcol, tsl], box["ps"][:],
                                    bqk_sb[:, jcol:jcol + 1], None,
                                    op0=mybir.AluOpType.add)
                    return step
                for i in range(nsteps):
                    steps.append(mk(i))
                return steps

            def v_group(tglob):
                tci, tt = tglob // 4, tglob % 4
                steps = []
                box = {}

                def mk(k):
                    first, last = k == 0, k == KT - 1

                    def step():
                        if first:
                            box["ps"] = psf.tile([P, 512], F32, tag="psf",
                                                 name=f"v_{tglob}")
                        nc.tensor.matmul(
                            box["ps"][:],
                            xt16_sb[:, tci, k, tt * P:(tt + 1) * P],
                            wv_sb[:, k, :],
                            start=first, stop=last)
                        if last:
                            nc.vector.tensor_tensor(
                                vaug[:, tglob, :, 0:HD],
                                box["ps"][:].rearrange(
                                    "p (h d) -> p h d", h=HPC),
                                bvb_sb[:].rearrange("p (h d) -> p h d", h=HPC),
                                op=mybir.AluOpType.add)
                    return step
                for k in range(KT):
                    steps.append(mk(k))
                return steps

            ost_box = {}

            def outproj_part_group(tt, cc):
                """jt 0..2 partial accumulation (needs heads 0..5 only)."""
                steps = []
                box = {}

                def mk(jt):
                    first, last = jt == 0, jt == 2

                    def step():
                        if first:
                            box["ps"] = psf.tile([P, 512], F32, tag="psf",
                                                 name=f"opp_{tt}_{cc}")
                        nc.tensor.matmul(
                            box["ps"][:],
                            ot_sb[:, jt, tt * P:(tt + 1) * P],
                            wo_sb[:, jt, cc * 512:(cc + 1) * 512],
                            start=first, stop=last)
                        if last:
                            nc.vector.tensor_copy(
                                ost_part[:, tt, cc * 512:(cc + 1) * 512],
                                box["ps"][:])
                    return step
                for jt in range(3):
                    steps.append(mk(jt))
                return steps

            def outproj_final_group(tt, cc, drain=False):
                """jt 3 matmul (heads 6,7) + add of the jt0-2 partial.

                In the drain, the partial is instead pre-loaded into the
                psum through an identity matmul and the sum is evacuated by
                the (idle) scalar engine, keeping the tail off the DVE."""
                steps = []

                def step():
                    ps = psf.tile([P, 512], F32, tag="psf",
                                  name=f"opf_{tt}_{cc}")
                    if cc == 0:
                        ost_box[tt] = ostpool.tile(
                            [P, D], BF16, tag="ost", name=f"ost_{tt}")
                    if drain:
                        nc.tensor.matmul(
                            ps[:], ident_sb[:],
                            ost_part[:, tt, cc * 512:(cc + 1) * 512],
                            start=True, stop=False)
                    nc.tensor.matmul(
                        ps[:],
                        ot_sb[:, 3, tt * P:(tt + 1) * P],
                        wo_sb[:, 3, cc * 512:(cc + 1) * 512],
                        start=not drain, stop=True)
                    if drain:
                        nc.scalar.activation(
                            ost_box[tt][:, cc * 512:(cc + 1) * 512],
                            ps[:], COPY)
                    else:
                        nc.vector.tensor_tensor(
                            ost_box[tt][:, cc * 512:(cc + 1) * 512],
                            ps[:],
                            ost_part[:, tt, cc * 512:(cc + 1) * 512],
                            op=mybir.AluOpType.add)
                    if cc == 1:
                        eng = nc.sync if tt % 2 == 0 else nc.gpsimd
                        eng.dma_start(out_d[tt * P:(tt + 1) * P, :],
                                      ost_box[tt][:])
                steps.append(step)
                return steps

            # ---------------- preamble ----------------
            for tci in range(TCH):
                for s in kq_group(4, tci):   # kT j0 tile (heads 0,1 k)
                    s()
                for s in kq_group(0, tci):   # q j0 tile (heads 0,1 q)
                    s()
            for tg in range(4):
                for s in v_group(tg):
                    s()
            nc.vector.tensor_copy(qpads[0][0:HD, :], qk_sb[0:HD, 0, :])

            # ---------------- filler window assignments ----------------
            # win key = (h, half); each window spans 16 attention iters.
            win_steps = {}
            win_start = {}
            win_len = {}

            def assign(windows, groups, start=0, length=16):
                flat = [s for g in groups for s in g]
                n = len(windows)
                for i, w in enumerate(windows):
                    win_steps[w] = flat[len(flat) * i // n:
                                        len(flat) * (i + 1) // n]
                    win_start[w] = start
                    win_len[w] = length

            # v tiles 4..15 must land just ahead of AV(h0, half0, k)
            assign([(0, 0)], [v_group(tg) for tg in range(4, 16)], length=14)
            assign([(0, 1), (1, 0)],
                   [kq_group(5, tci) for tci in range(TCH)]
                   + [kq_group(1, tci) for tci in range(TCH)])
            assign([(1, 1), (2, 0)],
                   [kq_group(6, tci) for tci in range(TCH)]
                   + [kq_group(2, tci) for tci in range(TCH)])
            assign([(2, 1), (3, 0), (3, 1), (4, 0)],
                   [kq_group(7, tci) for tci in range(TCH)]
                   + [kq_group(3, tci) for tci in range(TCH)])
            # out-proj partials (jt 0-2) fill the otherwise-empty late
            # windows; tiles 0-7 (t columns of half 0) need heads 0-5 half0
            # (done by (5,1)), tiles 8-15 need half1 (done by (6,1)).
            assign([(5, 1)], [outproj_part_group(tt, cc)
                              for tt in range(0, 4) for cc in range(2)],
                   start=1)
            assign([(6, 0)], [outproj_part_group(tt, cc)
                              for tt in range(4, 8) for cc in range(2)])
            assign([(6, 1)], [outproj_part_group(tt, cc)
                              for tt in range(8, 12) for cc in range(2)])
            assign([(7, 0)], [outproj_part_group(tt, cc)
                              for tt in range(12, 16) for cc in range(2)])
            assign([(7, 1)],
                   [outproj_final_group(tt, cc) for tt in range(8)
                    for cc in range(2)], start=3, length=13)

            # ---------------- attention stream ----------------
            # Software pipeline: scores run one iteration ahead of exp so
            # the scalar engine never waits on the PE; AV trails by one
            # iteration; the previous half's psum evacuation is queued on
            # the scalar engine before the current exp so the AV psum bank
            # frees quickly.
            iters = [(h, half, k)
                     for h in range(HPC) for half in range(2)
                     for k in range(TT)]
            pss_tiles = {}
            af_tiles = {}
            sums_tiles = {}
            wt_tiles = {}

            def emit_scores(h, half, k):
                ps = pss.tile([P, 2, 512], F32, tag="pss",
                              name=f"s_{h}_{half}_{k}")
                kT2 = qk_sb[:, 4 + h // 2, :]
                qpad = qpads[h % 2]
                for c in range(2):
                    nc.tensor.matmul(
                        ps[:, c, :],
                        kT2[:, k * P:(k + 1) * P],
                        qpad[:, half * 1024 + c * 512:
                             half * 1024 + (c + 1) * 512],
                        start=True, stop=True)
                pss_tiles[(h, half, k)] = ps

            def emit_av(ph, phalf, pk):
                if pk == 0:
                    af_tiles[(ph, phalf)] = avp.tile(
                        [P, 2, 512], F32, tag="av", name=f"af_{ph}_{phalf}")
                paf = af_tiles[(ph, phalf)]
                pwt = wt_tiles.pop((ph, phalf, pk))
                for c in range(2):
                    nc.tensor.matmul(
                        paf[:, c, :],
                        vaug_f[:, pk, ph * (HD + 1):ph * (HD + 1) + P],
                        pwt[:, c * 512:(c + 1) * 512],
                        start=(pk == 0), stop=(pk == TT - 1))

            def emit_evac_finish(ph, phalf):
                paf = af_tiles.pop((ph, phalf))
                pb = (ph % 2) * 64
                jt = ph // 2
                hsl = slice(phalf * 1024, (phalf + 1) * 1024)
                if phalf == 0:
                    sums_tiles[ph] = nrmpool.tile([P, 1024], F32, tag="sums",
                                                  name=f"sums_{ph}")
                    nc.gpsimd.memset(sums_tiles[ph][0:64, :], 1.0)
                sums = sums_tiles[ph]
                spart = 32 * phalf
                # o rows on DVE; denominator row via scalar engine in one
                # [1,1024] copy (shares the exp act table -> no reload)
                nc.vector.tensor_copy(
                    ot_sb[pb:pb + HD, jt, hsl],
                    paf[0:HD].rearrange("p a b -> p (a b)"))
                nc.scalar.activation(
                    sums[spart:spart + 1, :],
                    paf[HD:HD + 1].rearrange("p a b -> p (a b)"), COPY)
                rcp = rcpool.tile([P, 1024], F32, tag="rcp",
                                  name=f"rcp_{ph}_{phalf}")
                nc.vector.reciprocal_approx_fast(
                    rcp[0:64, :], sums[0:64, :])
                if ph == HPC - 1:
                    # last head: broadcast 1/den through the PE (fp32 rank-1
                    # matmuls) to skip the DRAM-bounce latency on the
                    # critical tail
                    for c in range(2):
                        rbp = psf.tile([P, 512], F32, tag="psf",
                                       name=f"rbp_{phalf}_{c}")
                        nc.tensor.matmul(
                            rbp[pb:pb + 64, :],
                            ones64[spart:spart + 1, :],
                            rcp[spart:spart + 1, c * 512:(c + 1) * 512],
                            start=True, stop=True)
                        csl = slice(phalf * 1024 + c * 512,
                                    phalf * 1024 + (c + 1) * 512)
                        nc.vector.tensor_tensor(
                            ot_sb[pb:pb + 64, jt, csl],
                            ot_sb[pb:pb + 64, jt, csl],
                            rbp[pb:pb + 64, :], op=mybir.AluOpType.mult)
                else:
                    rd = rdpool.tile([2, 512], F32, tag="rd",
                                     name=f"rd_{ph}_{phalf}")
                    nc.sync.dma_start(
                        rd[:].rearrange("a b -> (a b)"),
                        rcp[spart:spart + 1, :])
                    rb = rbpool.tile([P, 1024], F32, tag="rb",
                                     name=f"rb_{ph}_{phalf}")
                    rd_bcast = bass.AP(
                        tensor=rd.tensor, offset=rd.offset,
                        ap=[[0, 64], [512, 2], [1, 512]])
                    nc.sync.dma_start(
                        rb[pb:pb + 64, :].rearrange("p (c r) -> p c r", c=2),
                        rd_bcast)
                    nc.vector.tensor_mul(
                        ot_sb[pb:pb + 64, jt, hsl],
                        ot_sb[pb:pb + 64, jt, hsl],
                        rb[pb:pb + 64, :])

            win_emitted = {w: 0 for w in win_steps}
            prev = None
            emit_scores(*iters[0])
            for gi, (h, half, k) in enumerate(iters):
                if gi + 1 < len(iters):
                    emit_scores(*iters[gi + 1])
                w = (h, half)
                steps = win_steps.get(w, [])
                if steps:
                    s0 = win_start.get(w, 0)
                    slen = win_len.get(w, 16)
                    if k >= s0:
                        tgt = len(steps) if k == TT - 1 else min(
                            len(steps),
                            -(-len(steps) * (k - s0 + 1) // slen)
                            + (3 if k == s0 else 0))
                        while win_emitted[w] < tgt:
                            steps[win_emitted[w]]()
                            win_emitted[w] += 1
                if prev is not None and prev[2] == TT - 1:
                    emit_av(*prev)
                    emit_evac_finish(prev[0], prev[1])
                wt = wtpool.tile([P, 1024], BF16, tag="wt",
                                 name=f"wt_{h}_{half}_{k}")
                nc.scalar.activation(
                    wt[:], pss_tiles.pop((h, half, k))[:].rearrange(
                        "p a b -> p (a b)"),
                    EXP, bias=0.0, scale=SC)
                wt_tiles[(h, half, k)] = wt
                if prev is not None and prev[2] != TT - 1:
                    emit_av(*prev)
                prev = (h, half, k)
                # prefetch next head's qpad late in this head
                if half == 1 and k == 12 and h + 1 < HPC:
                    nh = h + 1
                    npb = (nh % 2) * 64
                    nc.vector.tensor_copy(
                        qpads[nh % 2][npb:npb + HD, :],
                        qk_sb[npb:npb + HD, nh // 2, :])

            # drain: last AV + finish + outproj finals for tiles 8..15
            emit_av(*prev)
            emit_evac_finish(prev[0], prev[1])
            for tt in range(8, 16):
                for cc in range(2):
                    for s in outproj_final_group(tt, cc, drain=True):
                        s()

    nc.compile()
    return nc


def _prep_core_inputs(x, qkv_w, qkv_b, out_w, core):
    b, g = core // 2, core % 2
    jsl = slice(g * JC, (g + 1) * JC)

    xT = np.ascontiguousarray(x[b].T)                       # [1024, 2048]
    xk = xT.reshape(KT, P, TCH, 512)
    xt16 = np.ascontiguousarray(xk.transpose(2, 1, 0, 3))   # [4,128,8,512]

    Wq = qkv_w[0 * D:1 * D][jsl]                            # [512, 1024]
    Wk = qkv_w[1 * D:2 * D][jsl]
    Wv = qkv_w[2 * D:3 * D][jsl]
    WqkT = np.concatenate([Wq, Wk], axis=0).T               # [1024, 1024]
    WvT = Wv.T                                              # [1024, 512]
    wv = np.ascontiguousarray(
        WvT.reshape(KT, P, JC).transpose(1, 0, 2))          # [128, 8, 512]

    bq = qkv_b[0 * D:1 * D][jsl]
    bk = qkv_b[1 * D:2 * D][jsl]
    bv = qkv_b[2 * D:3 * D][jsl]
    bqk = np.ascontiguousarray(
        np.concatenate([bq, bk]).reshape(8, P).T)           # [128, 8]
    bvb = np.ascontiguousarray(np.tile(bv[None, :], (P, 1)))

    WoT = np.ascontiguousarray(out_w[:, jsl].T)             # [512, 1024]
    wo = np.ascontiguousarray(
        WoT.reshape(JC // P, P, D).transpose(1, 0, 2))      # [128, 4, 1024]

    inputs = {
        "xt16": xt16.astype(BF16_NP),
        "ident": np.eye(P, dtype=BF16_NP),
        "wv": wv.astype(BF16_NP),
        "wo": wo.astype(BF16_NP),
        "bqk": bqk.astype(np.float32),
        "bvb": bvb.astype(BF16_NP),
    }
    if USE_FP8_KQ:
        xt8 = np.ascontiguousarray(
            (xk * X8_SCALE).reshape(KT // 2, 2, P, TCH, 512)
            .transpose(3, 2, 0, 1, 4))                      # [4,128,4,2,512]
        wqk8 = np.ascontiguousarray(
            (WqkT * W8_SCALE).reshape(KT // 2, 2, P, 8, P)
            .transpose(2, 3, 0, 1, 4))                      # [128,8,4,2,128]
        inputs["xt8"] = xt8.astype(FP8_NP)
        inputs["wqk"] = wqk8.astype(FP8_NP)
    else:
        wqk = np.ascontiguousarray(
            WqkT.reshape(KT, P, 2 * JC).transpose(1, 0, 2))  # [128, 8, 1024]
        inputs["wqk"] = wqk.astype(BF16_NP)
    return inputs


def run(x, qkv_w, qkv_b, out_w, out_b, trace=False, tmpdir=None):
    if "nc" not in _cached:
        _cached["nc"] = build_program()
    nc = _cached["nc"]
    in_maps = [_prep_core_inputs(x, qkv_w, qkv_b, out_w, c) for c in range(NC)]
    res = run_bass_kernel_spmd(nc, in_maps, core_ids=list(range(NC)),
                               trace=trace, tmpdir=tmpdir)
    parts = np.stack([np.asarray(res.results[c]["out"], dtype=np.float32)
                      for c in range(NC)])                  # [8, T, D]
    out = parts.reshape(B, 2, T, D).sum(axis=1) + out_b[None, None, :]
    return out.astype(np.float32), res


def kernel(x, qkv_w, qkv_b, out_w, out_b):
    x = np.asarray(x, dtype=np.float32)
    qkv_w = np.asarray(qkv_w, dtype=np.float32)
    qkv_b = np.asarray(qkv_b, dtype=np.float32)
    out_w = np.asarray(out_w, dtype=np.float32)
    out_b = np.asarray(out_b, dtype=np.float32)
    out, _ = run(x, qkv_w, qkv_b, out_w, out_b, trace=False)
    return out


if __name__ == "__main__":
    import jax
    import reference
    inputs = {k: np.asarray(v) for k, v in reference.setup_inputs().items()}
    expected = np.asarray(reference.reference(**inputs))
    got = kernel(**inputs)
    err = np.linalg.norm(got - expected) / np.linalg.norm(expected)
    print("Relative error:", err)
